# revision 1
# baseline (speedup 1.0000x reference)
"""NexusNet GNN message-passing kernel v2 for 8 Trainium2 NeuronCores.

Sharding:
  - nexus_up + nexus MLP: sharded by nexus node (M/8 contiguous segs/core);
    edges routed to the core owning their dst segment (host index prep).
    x gathered from a bf16 table via 128-row indirect DMAs; aggregation via
    one-hot matmul on PE into PSUM per 128-seg block.
  - n [M,176] f32 rows (160 n + 15 b + pad) AllGathered to every core.
  - down: sharded by planar node (N/8 per core, 2 halves/core/plane), fused
    A+B+C per (plane,half):
      A: x loaded feature-major into ft (also the down-MLP input); per-node
         edge-logit a values + invdeg computed on PE into an SBUF table.
      B: per 128-src block (K_B 128-slot groups): indirect-gather n rows of
         the block's edges, broadcast a to slots via range-mask matmul,
         per-slot softmax weights (invdeg folded), one-hot matmul aggregates
         messages feature-major straight into ft.
      C: 2-layer down MLP feature-major on PE; output transposed on host.
"""

import numpy as np

import concourse.bass as bass
import concourse.bacc as bacc
import concourse.mybir as mybir
import concourse.tile as tile

F32 = mybir.dt.float32
F32R = mybir.dt.float32r
BF16 = mybir.dt.bfloat16
I32 = mybir.dt.int32
TANH = mybir.ActivationFunctionType.Tanh
EXP = mybir.ActivationFunctionType.Exp
ALU = mybir.AluOpType

CFG_FULL = dict(P=3, N=100000, M=30000, E=200000, C=5, FP=64, FN=32, NC=8)

GRP = 4               # up-phase seg blocks per nexus-MLP group
CHW = 512             # down-MLP chunk width (4 src blocks)
TB = 3                # stage-B src blocks per batched iteration
NR2 = 176             # n-row floats (160 n + 15 b + 1 pad)


def _ceil(a, b):
    return (a + b - 1) // b


def host_prep(inputs, cfg):
    P, N, M, E, C, FP, FN, NC = (cfg[k] for k in
                                 ("P", "N", "M", "E", "C", "FP", "FN", "NC"))
    CF = C * FP
    M_LOC = M // NC
    N_LOC = N // NC
    NH = N_LOC // 2                       # nodes per half
    NHP = _ceil(NH, 128) * 128            # padded half
    NMT = NHP // 128                      # src blocks per half
    NB = _ceil(M_LOC, 128)                # up seg blocks per core

    x = np.ascontiguousarray(np.asarray(inputs["x"], np.float32)
                             .reshape(P, N, CF))
    esrc = np.asarray(inputs["edge_src"])
    edst = np.asarray(inputs["edge_dst"])

    bfnp = mybir.dt.np(BF16)
    xbf = np.ascontiguousarray(x.reshape(P * N, CF)).astype(bfnp)

    # xloc is built after the relabel below (feature-major [P, 2, CF, NH])

    # ---------------- UP phase indices ----------------
    per_kp = {}
    max_blk_cnt = 0
    for p in range(P):
        order = np.argsort(edst[p], kind="stable")
        ds, ss = edst[p][order], esrc[p][order]
        bounds = np.searchsorted(ds, np.arange(NC + 1) * M_LOC)
        for k in range(NC):
            sl = slice(bounds[k], bounds[k + 1])
            dsl = (ds[sl] - k * M_LOC).astype(np.int64)
            blk = dsl >> 7
            cnt = np.bincount(blk, minlength=NB)
            max_blk_cnt = max(max_blk_cnt, int(cnt.max(initial=0)))
            per_kp[(k, p)] = (dsl, (ss[sl] + p * N).astype(np.int64), blk, cnt)
    K_UP = max(1, _ceil(max_blk_cnt, 128))
    NBK = NB * K_UP

    up_src = np.zeros((NC, P, NBK * 128), np.int32)
    up_dr = np.full((NC, P, NBK * 128), -1.0, np.float32)
    # (converted to bf16 below; values 0..128 are exact)
    for (k, p), (dsl, sglob, blk, cnt) in per_kp.items():
        starts = np.concatenate(([0], np.cumsum(cnt)))[:-1]
        r = np.arange(len(dsl)) - np.repeat(starts, cnt)
        pos = blk * (K_UP * 128) + r
        up_src[k, p, pos] = sglob
        up_dr[k, p, pos] = dsl - (blk << 7)
    up_src = up_src.reshape(NC, P, NBK, 128).transpose(0, 1, 3, 2).copy()
    up_dr = (up_dr.reshape(NC, P, NBK, 128).transpose(0, 1, 3, 2)
             .astype(bfnp).copy())

    # ---------------- DOWN (stage B) indices ----------------
    # Per-plane striped degree-sort relabel: sorted rank r -> core r%NC,
    # local slot r//NC (-> half, loc). Per 128-src block, a per-block group
    # count profile Kt[ph][t] = max over cores ceil(cnt/128); slot-group
    # columns packed at goff[ph][t].
    perm_core = np.empty((P, N), np.int64)
    perm_h = np.empty((P, N), np.int64)
    perm_loc = np.empty((P, N), np.int64)
    edges = {}
    for p in range(P):
        degp = np.bincount(esrc[p], minlength=N)
        orderp = np.argsort(-degp, kind="stable")
        rank = np.empty(N, np.int64)
        rank[orderp] = np.arange(N)
        loc = rank // NC
        perm_core[p] = rank % NC
        perm_h[p] = loc // NH
        perm_loc[p] = loc % NH
        ec, eh = perm_core[p][esrc[p]], perm_h[p][esrc[p]]
        el, ed = perm_loc[p][esrc[p]], edst[p].astype(np.int64)
        for k in range(NC):
            for h in range(2):
                sel = (ec == k) & (eh == h)
                s_loc = el[sel]
                d_loc = ed[sel]
                o2 = np.argsort(s_loc, kind="stable")
                edges[(k, p, h)] = (s_loc[o2], d_loc[o2])
    # chunk-major n_full layout: AllGather chunk j (CH local rows) lands
    # contiguously at NC*CH*j; nexus id m -> (core m//M_LOC, loc m%M_LOC)
    CH = 2 * GRP * 128
    J_AG = _ceil(M_LOC, CH)
    lastch = M_LOC - (J_AG - 1) * CH

    def g2r(m):
        k_ = m // M_LOC
        r_ = m % M_LOC
        j_ = np.minimum(r_ // CH, J_AG - 1)
        base = NC * CH * j_
        chs = np.where(j_ < J_AG - 1, CH, lastch)
        return base + k_ * chs + (r_ - CH * j_)

    # profiles
    Kt = np.zeros((2 * P, NMT), np.int64)
    cnts = {}
    for (k, p, h), (s_loc, d_loc) in edges.items():
        ph = p * 2 + h
        cnt = np.bincount(s_loc >> 7, minlength=NMT)
        cnts[(k, p, h)] = cnt
        Kt[ph] = np.maximum(Kt[ph], (cnt + 127) // 128)
    goff = np.zeros((2 * P, NMT + 1), np.int64)
    goff[:, 1:] = np.cumsum(Kt, axis=1)
    G2 = int(goff[:, -1].max())

    bdst = np.zeros((NC, 2 * P, 128, G2), np.int32)
    bsrel = np.full((NC, 2 * P, 128, G2), -1.0, bfnp)
    bstart = np.zeros((NC, 2 * P, 128, G2), bfnp)
    bend = np.zeros((NC, 2 * P, 128, G2), bfnp)
    degt = np.ones((NC, 2 * P, 128, NMT), np.float32)
    for (k, p, h), (s_loc, d_loc) in edges.items():
        ph = p * 2 + h
        cnt = cnts[(k, p, h)]
        deg = np.bincount(s_loc, minlength=NHP)
        degt[k, ph, :, :] = np.maximum(
            deg.reshape(NMT, 128).T, 1.0).astype(np.float32)
        bb = np.concatenate(([0], np.cumsum(cnt)))
        for t in range(NMT):
            seg_s = s_loc[bb[t]:bb[t + 1]] - 128 * t
            seg_d = d_loc[bb[t]:bb[t + 1]]
            n_t = len(seg_s)
            cum = np.concatenate(
                ([0], np.cumsum(np.bincount(seg_s, minlength=128))))
            for jj in range(int(Kt[ph][t])):
                g = int(goff[ph][t]) + jj
                lo = jj * 128
                nh_ = min(max(n_t - lo, 0), 128)
                if nh_ > 0:
                    bdst[k, ph, :nh_, g] = g2r(seg_d[lo:lo + nh_])
                    bsrel[k, ph, :nh_, g] = seg_s[lo:lo + nh_]
                bstart[k, ph, :, g] = np.clip(cum[:128] - lo, 0, 128)
                bend[k, ph, :, g] = np.clip(cum[1:] - lo, 0, 128)

    # per-core feature-major x slices after relabel: [NC, P, 2, CF, NH]
    inv = np.empty((P, NC, 2, NH), np.int64)
    for p in range(P):
        inv[p, perm_core[p], perm_h[p], perm_loc[p]] = np.arange(N)
    xloc = np.empty((NC, P, 2, CF, NH), np.float32)
    for p in range(P):
        for k in range(NC):
            for h in range(2):
                xloc[k, p, h] = x[p][inv[p, k, h]].T

    # ---------------- weights ----------------
    g = lambda n: np.asarray(inputs[n], np.float32)
    Wn1, Wn2, We, Wd1, Wd2 = g("Wn1"), g("Wn2"), g("We"), g("Wd1"), g("Wd2")
    bn1, bn2, be, bd1, bd2 = g("bn1"), g("bn2"), g("be"), g("bd1"), g("bd2")

    wn1t = np.stack([Wn1.transpose(2, 0, 1)[p * FP:(p + 1) * FP]
                     .reshape(FP, C * FN) for p in range(P)]).copy()
    wn2t = Wn2.transpose(2, 0, 1).reshape(FN, C * FN).copy()
    went = We[:, :, 0, FP:]                                   # [P, C, FN]
    wentA = np.zeros((4 * FN, C * P), np.float32)
    for c in range(4):
        wentA[c * FN:(c + 1) * FN, c * P:(c + 1) * P] = went[:, c, :].T
    wentB = np.zeros((FN + 1, C * P), np.float32)
    wentB[:FN, 4 * P:] = went[:, 4, :].T
    wentB[FN, :] = be[:, :, 0].T.reshape(-1)
    bn1c = bn1.reshape(C, FN, 1).copy()
    bn2c = bn2.reshape(C, FN, 1).copy()
    we1 = We[:, :, 0, :FP].transpose(0, 2, 1).copy()          # [P, FP, C]
    wd1t = Wd1.transpose(0, 3, 1, 2).reshape(P, FP + FN, C * FP).copy()
    wd2t = Wd2.transpose(0, 1, 3, 2).copy()                   # [P, C, FP, FP]
    bd1c = bd1.reshape(P, C, FP, 1).copy()
    bd2c = bd2.reshape(P, C, FP, 1).copy()
    iota = np.tile(np.arange(128, dtype=np.float32), (128, 1)).copy()
    iotab = iota.astype(bfnp)
    ident = np.eye(128, dtype=np.float32)
    identb = np.eye(128, dtype=np.float32).astype(bfnp)

    meta = dict(cfg=cfg, M_LOC=M_LOC, N_LOC=N_LOC, NH=NH, NHP=NHP,
                NMT=NMT, NB=NB, K_UP=K_UP, NBK=NBK, G2=G2,
                Kt=tuple(tuple(int(v) for v in row) for row in Kt),
                goff=tuple(tuple(int(v) for v in row) for row in goff),
                perm_core=perm_core, perm_h=perm_h, perm_loc=perm_loc)

    shared = dict(xbf=xbf, wn1t=wn1t, wn2t=wn2t, wentA=wentA, wentB=wentB,
                  bn1c=bn1c, bn2c=bn2c, we1=we1, wd1t=wd1t, wd2t=wd2t,
                  bd1c=bd1c, bd2c=bd2c, iota=iota, iotab=iotab,
                  ident=ident, identb=identb)
    in_maps = []
    for k in range(NC):
        m = dict(shared)
        m.update(xloc=xloc[k], up_src=up_src[k], up_dr=up_dr[k],
                 bdst=bdst[k], bsrel=bsrel[k], bstart=bstart[k],
                 bend=bend[k], degt=degt[k])
        in_maps.append(m)
    return in_maps, meta


def build_kernel(meta, stages=("up", "ag", "a", "b", "c")):
    stages = set(stages)
    cfg = meta["cfg"]
    P, N, M, E, C, FP, FN, NC = (cfg[k] for k in
                                 ("P", "N", "M", "E", "C", "FP", "FN", "NC"))
    M_LOC, NH, NHP, NMT = meta["M_LOC"], meta["NH"], meta["NHP"], meta["NMT"]
    NB, K_UP, NBK = meta["NB"], meta["K_UP"], meta["NBK"]
    G2 = meta["G2"]
    Kt, goff = meta["Kt"], meta["goff"]
    K_MAX = max(max(row) for row in Kt)
    CF = C * FP
    CN = C * FN
    assert C == 5

    nc = bacc.Bacc("TRN2", num_devices=NC)

    def param(name, shape, dt=F32, out=False):
        return nc.declare_dram_parameter(name, list(shape), dt, isOutput=out)

    xbf_d = param("xbf", [P * N, CF], BF16)
    xloc_d = param("xloc", [P, 2, CF, NH])
    up_src_d = param("up_src", [P, 128, NBK], I32)
    up_dr_d = param("up_dr", [P, 128, NBK], BF16)
    bdst_d = param("bdst", [2 * P, 128, G2], I32)
    bsrel_d = param("bsrel", [2 * P, 128, G2], BF16)
    bstart_d = param("bstart", [2 * P, 128, G2], BF16)
    bend_d = param("bend", [2 * P, 128, G2], BF16)
    degt_d = param("degt", [2 * P, 128, NMT])
    wn1t_d = param("wn1t", [P, FP, CN])
    wn2t_d = param("wn2t", [FN, CN])
    wentA_d = param("wentA", [4 * FN, C * P])
    wentB_d = param("wentB", [FN + 1, C * P])
    bn1c_d = param("bn1c", [C, FN, 1])
    bn2c_d = param("bn2c", [C, FN, 1])
    we1_d = param("we1", [P, FP, C])
    wd1t_d = param("wd1t", [P, FP + FN, C * FP])
    wd2t_d = param("wd2t", [P, C, FP, FP])
    bd1c_d = param("bd1c", [P, C, FP, 1])
    bd2c_d = param("bd2c", [P, C, FP, 1])
    iota_d = param("iota", [128, 128])
    iotab_d = param("iotab", [128, 128], BF16)
    ident_d = param("ident", [128, 128])
    identb_d = param("identb", [128, 128], BF16)
    out_d = param("outT", [P, 2, C, FP, NHP], out=True)

    n_loc = nc.dram_tensor("n_loc", [M_LOC, NR2], F32)
    n_full = nc.dram_tensor("n_full", [NC * M_LOC, NR2], F32,
                            addr_space="Shared")

    with tile.TileContext(nc) as tc:
        with tc.tile_pool(name="const", bufs=1) as cp:
            iota_t = cp.tile([128, 128], F32)
            nc.sync.dma_start(out=iota_t[:], in_=iota_d[:])
            iotab_t = cp.tile([128, 128], BF16)
            nc.sync.dma_start(out=iotab_t[:], in_=iotab_d[:])
            ident_t = cp.tile([128, 128], F32)
            nc.sync.dma_start(out=ident_t[:], in_=ident_d[:])
            identb_t = cp.tile([128, 128], BF16)
            nc.sync.dma_start(out=identb_t[:], in_=identb_d[:])
            wn1t_t = [cp.tile([FP, CN], F32R, name=f"wn1t{p}")
                      for p in range(P)]
            wn2t_t = cp.tile([FN, CN], F32R)
            wentA_t = cp.tile([4 * FN, C * P], F32R)
            wentB_t = cp.tile([FN + 1, C * P], F32R)
            nc.sync.dma_start(out=wn2t_t[:], in_=wn2t_d[:].bitcast(F32R))
            nc.sync.dma_start(out=wentA_t[:], in_=wentA_d[:].bitcast(F32R))
            nc.sync.dma_start(out=wentB_t[:], in_=wentB_d[:].bitcast(F32R))
            bn1c_t = [cp.tile([FN, 1], F32, name=f"bn1c{c}") for c in range(C)]
            bn2c_t = [cp.tile([FN, 1], F32, name=f"bn2c{c}") for c in range(C)]
            we1_t = [cp.tile([FP, C], F32, name=f"we1{p}") for p in range(P)]
            wd1t_t = [cp.tile([FP + FN, C * FP], F32R, name=f"wd1t{p}")
                      for p in range(P)]
            wd2t_t = [[cp.tile([FP, FP], F32R, name=f"wd2t{p}_{c}")
                       for c in range(C)] for p in range(P)]
            bd1c_t = [[cp.tile([FP, 1], F32, name=f"bd1c{p}_{c}")
                       for c in range(C)] for p in range(P)]
            bd2c_t = [[cp.tile([FP, 1], F32, name=f"bd2c{p}_{c}")
                       for c in range(C)] for p in range(P)]
            for p in range(P):
                nc.sync.dma_start(out=wn1t_t[p][:], in_=wn1t_d[p].bitcast(F32R))
                nc.sync.dma_start(out=we1_t[p][:], in_=we1_d[p])
                nc.sync.dma_start(out=wd1t_t[p][:], in_=wd1t_d[p].bitcast(F32R))
                for c in range(C):
                    nc.sync.dma_start(out=wd2t_t[p][c][:],
                                      in_=wd2t_d[p, c].bitcast(F32R))
                    nc.sync.dma_start(out=bd1c_t[p][c][:], in_=bd1c_d[p, c])
                    nc.sync.dma_start(out=bd2c_t[p][c][:], in_=bd2c_d[p, c])
            for c in range(C):
                nc.sync.dma_start(out=bn1c_t[c][:], in_=bn1c_d[c])
                nc.sync.dma_start(out=bn2c_t[c][:], in_=bn2c_d[c])
            upsrc_t = [cp.tile([128, NBK], I32, name=f"upsrc{p}")
                       for p in range(P)]
            updr_t = [cp.tile([128, NBK], BF16, name=f"updr{p}")
                      for p in range(P)]
            for p in range(P):
                nc.scalar.dma_start(out=upsrc_t[p][:], in_=up_src_d[p])
                nc.scalar.dma_start(out=updr_t[p][:], in_=up_dr_d[p])
            bdst_t, bsrel_t, bstart_t, bend_t, degt_t = [], [], [], [], []
            for ph in range(2 * P):
                d = cp.tile([128, G2], I32, name=f"bdst{ph}")
                nc.scalar.dma_start(out=d[:], in_=bdst_d[ph])
                bdst_t.append(d)
                s = cp.tile([128, G2], BF16, name=f"bsrel{ph}")
                nc.scalar.dma_start(out=s[:], in_=bsrel_d[ph])
                bsrel_t.append(s)
                s0 = cp.tile([128, G2], BF16, name=f"bstart{ph}")
                nc.scalar.dma_start(out=s0[:], in_=bstart_d[ph])
                bstart_t.append(s0)
                s1 = cp.tile([128, G2], BF16, name=f"bend{ph}")
                nc.scalar.dma_start(out=s1[:], in_=bend_d[ph])
                bend_t.append(s1)
                dg = cp.tile([128, NMT], F32, name=f"degt{ph}")
                nc.scalar.dma_start(out=dg[:], in_=degt_d[ph])
                degt_t.append(dg)
            ones_f = cp.tile([1, GRP * 128], F32)
            nc.vector.memset(ones_f[:], 1.0)
            ones_r = cp.tile([1, GRP * 128], F32R)
            nc.vector.tensor_copy(out=ones_r[:], in_=ones_f[:])

            # ======================= UP PHASE =======================
            n_loc_ap = n_loc.ap()
            with tc.tile_pool(name="up_sb", bufs=3) as up, \
                 tc.tile_pool(name="up_sb1", bufs=2) as up1, \
                 tc.tile_pool(name="up_ps", bufs=2, space="PSUM") as upp, \
                 tc.tile_pool(name="up_ps1", bufs=1, space="PSUM") as upp1, \
                 tc.tile_pool(name="mlp_ps", bufs=1, space="PSUM") as mpp:
                for g0 in (range(0, NB, GRP) if "up" in stages else []):
                    gb = list(range(g0, min(g0 + GRP, NB)))
                    GW = len(gb) * 128
                    upX = [[up1.tile([FP, GRP * 128], F32R,
                                     name=f"upX{p}_{c}", tag=f"upX{p}_{c}")
                            for c in range(C)] for p in range(P)]
                    for p in range(P):
                        for bi, b in enumerate(gb):
                            O8 = up.tile([128, K_UP, 128], BF16, tag="O8")
                            csl0 = slice(b * K_UP, (b + 1) * K_UP)
                            nc.vector.tensor_tensor(
                                out=O8[:],
                                in0=updr_t[p][:, csl0]
                                    .rearrange("a (b c) -> a b c", c=1)
                                    .to_broadcast([128, K_UP, 128]),
                                in1=iotab_t[:].rearrange("a (b c) -> a b c",
                                                         b=1)
                                    .to_broadcast([128, K_UP, 128]),
                                op=ALU.is_equal)
                            pu = upp.tile([128, CF], F32, tag="pu",
                                          space="PSUM")
                            for kk in range(K_UP):
                                col = b * K_UP + kk
                                G = up.tile([128, CF], BF16, tag="G")
                                nc.gpsimd.indirect_dma_start(
                                    out=G[:], out_offset=None,
                                    in_=xbf_d[:],
                                    in_offset=bass.IndirectOffsetOnAxis(
                                        ap=upsrc_t[p][:, col:col + 1], axis=0))
                                nc.tensor.matmul(out=pu[:],
                                                 lhsT=O8[:, kk, :],
                                                 rhs=G[:], start=(kk == 0),
                                                 stop=(kk == K_UP - 1))
                            stg = up.tile([128, CF], F32, tag="stg")
                            nc.scalar.copy(out=stg[:], in_=pu[:])
                            csl = slice(bi * 128, (bi + 1) * 128)
                            for ti in range(3):
                                w = min(128, CF - ti * 128)
                                pt = upp1.tile([128, 128], F32, tag="ptr",
                                               space="PSUM")
                                nc.tensor.transpose(
                                    out=pt[:w, :],
                                    in_=stg[:, ti * 128:ti * 128 + w],
                                    identity=ident_t[:])
                                nc.vector.tensor_copy(
                                    out=upX[p][2 * ti][:, csl],
                                    in_=pt[0:FP, :])
                                if 2 * ti + 1 < C:
                                    nc.vector.tensor_copy(
                                        out=upX[p][2 * ti + 1][:, csl],
                                        in_=pt[FP:2 * FP, :])
                    # ---- nexus MLP over this group ----
                    n1c = [up.tile([FN, GRP * 128], F32R, name=f"n1c{c}",
                                   tag=f"n1c{c}") for c in range(C)]
                    for c in range(C):
                        pn1 = mpp.tile([FN, GRP * 128], F32, tag="pn1",
                                       space="PSUM", bufs=2)
                        for p in range(P):
                            nc.tensor.matmul(
                                out=pn1[:, :GW],
                                lhsT=wn1t_t[p][:, c * FN:(c + 1) * FN],
                                rhs=upX[p][c][:, :GW],
                                start=(p == 0), stop=(p == P - 1))
                        nc.scalar.activation(n1c[c][:, :GW], pn1[:, :GW],
                                             TANH, bias=bn1c_t[c][:])
                    n2s = up.tile([4 * FN, GRP * 128], F32R, tag="n2s")
                    nbt = up.tile([FN + 1, GRP * 128], F32R, tag="nbt")
                    nc.vector.tensor_copy(out=nbt[FN:FN + 1, :],
                                          in_=ones_r[:])
                    for c in range(C):
                        pn2 = mpp.tile([FN, GRP * 128], F32, tag="pn2",
                                       space="PSUM", bufs=2)
                        nc.tensor.matmul(
                            out=pn2[:, :GW],
                            lhsT=wn2t_t[:, c * FN:(c + 1) * FN],
                            rhs=n1c[c][:, :GW], start=True, stop=True)
                        dst = (n2s[c * FN:(c + 1) * FN, :GW] if c < 4
                               else nbt[0:FN, :GW])
                        nc.scalar.activation(dst, pn2[:, :GW],
                                             TANH, bias=bn2c_t[c][:])
                    pbv = mpp.tile([C * P, GRP * 128], F32, tag="misc",
                                   space="PSUM", bufs=1)
                    nc.tensor.matmul(out=pbv[:, :GW], lhsT=wentA_t[:],
                                     rhs=n2s[:, :GW], start=True, stop=False)
                    nc.tensor.matmul(out=pbv[:, :GW], lhsT=wentB_t[:],
                                     rhs=nbt[:, :GW], start=False, stop=True)
                    bt = up.tile([C * P, GRP * 128], F32, tag="bt")
                    nc.scalar.copy(out=bt[:, :GW], in_=pbv[:, :GW])
                    # assemble + store n rows per block
                    for bi, b in enumerate(gb):
                        rows = min(128, M_LOC - b * 128)
                        sl = slice(bi * 128, bi * 128 + 128)
                        tp = mpp.tile([128, NR2 - 1], F32,
                                      tag="misc", space="PSUM", bufs=1)
                        nc.tensor.transpose(
                            out=tp[:, 0:4 * FN],
                            in_=n2s[:, sl].bitcast(F32),
                            identity=ident_t[:])
                        nc.tensor.transpose(
                            out=tp[:, 4 * FN:CN],
                            in_=nbt[0:FN, sl].bitcast(F32),
                            identity=ident_t[:FN, :FN])
                        nc.tensor.transpose(
                            out=tp[:, CN:CN + C * P],
                            in_=bt[:, sl],
                            identity=ident_t[:C * P, :C * P])
                        nrow = up.tile([128, NR2], F32, tag="nrow")
                        nc.scalar.copy(out=nrow[:, 0:NR2 - 1], in_=tp[:])
                        nc.vector.memset(nrow[:, NR2 - 1:], 0.0)
                        nc.sync.dma_start(
                            out=n_loc_ap[b * 128:b * 128 + rows, :],
                            in_=nrow[:rows, :])
                    if "ag" in stages and (g0 // GRP) % 2 == 1 or \
                            ("ag" in stages and g0 + GRP >= NB):
                        CH = 2 * GRP * 128
                        jch = g0 // (2 * GRP)
                        lo = jch * CH
                        hi = min(lo + CH, M_LOC)
                        if hi > lo:
                            base = NC * CH * jch
                            nfv = n_full.ap()[base:base + NC * (hi - lo), :]
                            nc.gpsimd.collective_compute(
                                "AllGather", ALU.bypass,
                                replica_groups=[list(range(NC))],
                                ins=[n_loc_ap[lo:hi, :].opt()],
                                outs=[nfv.opt()])

            # ================= AllGather n (ablation fallback) ==========
            if "ag" in stages and "up" not in stages:
                CH = 2 * GRP * 128
                J_AG = _ceil(M_LOC, CH)
                for jch in range(J_AG):
                    lo = jch * CH
                    hi = min(lo + CH, M_LOC)
                    base = NC * CH * jch
                    nc.gpsimd.collective_compute(
                        "AllGather", ALU.bypass,
                        replica_groups=[list(range(NC))],
                        ins=[n_loc.ap()[lo:hi, :].opt()],
                        outs=[n_full.ap()[base:base + NC * (hi - lo), :]
                              .opt()])

            # ============ FUSED A+B+C per (plane, half) ============
            # stage-B iteration packing: <= NG groups and <= TBMAX blocks
            NG = max(10, K_MAX)
            TBMAX = 4
            packs = {}                    # ph -> list of (t0, tbw)
            for ph0 in range(2 * P):
                lst = []
                t = 0
                while t < NMT:
                    tw, gsum = 0, 0
                    while (t + tw < NMT and tw < TBMAX
                           and (tw == 0
                                or gsum + Kt[ph0][t + tw] <= NG)):
                        gsum += Kt[ph0][t + tw]
                        tw += 1
                    lst.append((t, tw))
                    t += tw
                packs[ph0] = lst
            with tc.tile_pool(name="ft_sb", bufs=1) as ftp, \
                 tc.tile_pool(name="ab_sb", bufs=2) as ab, \
                 tc.tile_pool(name="gn_sb", bufs=3) as gnp, \
                 tc.tile_pool(name="b_sb", bufs=2) as sbp, \
                 tc.tile_pool(name="b_ps", bufs=1, space="PSUM") as bps, \
                 tc.tile_pool(name="agg_ps", bufs=2, space="PSUM") as agp, \
                 tc.tile_pool(name="c_sb", bufs=2) as scb, \
                 tc.tile_pool(name="c_ps", bufs=1, space="PSUM") as scp:
                for ph in range(2 * P):
                    p, h = ph // 2, ph % 2
                    # ---- stage A: load x into ft, a-table ----
                    ft = [ftp.tile([FP + FN, NHP], F32R, name=f"ft{c}",
                                   tag=f"ft{c}") for c in range(C)]
                    a_sb = ab.tile([128, NMT, 8], BF16, tag="a_sb")
                    if "a" in stages:
                        for c in range(C):
                            if NHP > NH:
                                nc.vector.memset(
                                    ft[c][0:FP, NH:].bitcast(F32), 0.0)
                            nc.sync.dma_start(
                                out=ft[c][0:FP, :NH],
                                in_=xloc_d[p, h, c * FP:(c + 1) * FP, :]
                                    .bitcast(F32R))
                        with nc.allow_low_precision(reason="invdeg bf16"):
                            nc.vector.reciprocal(
                                out=a_sb[:, :, 5:6],
                                in_=degt_t[ph][:]
                                .rearrange("a (b c) -> a b c", c=1))
                        for t0 in range(0, NMT, 8):
                            tw = min(8, NMT - t0)
                            pa = bps.tile([128, 8, 8], F32, tag="pa",
                                          space="PSUM")
                            for ti in range(tw):
                                t = t0 + ti
                                tsl = slice(t * 128, (t + 1) * 128)
                                for c in range(C):
                                    nc.tensor.matmul(
                                        out=pa[:, ti, c:c + 1],
                                        lhsT=ft[c][0:FP, tsl].bitcast(F32),
                                        rhs=we1_t[p][:, c:c + 1],
                                        start=True, stop=True)
                            nc.vector.tensor_copy(
                                out=a_sb[:, t0:t0 + tw, 0:5],
                                in_=pa[:, 0:tw, 0:5])
                    # ---- stage B ----
                    if "b" in stages:
                        for (t0, tbw) in packs[ph]:
                            g0 = goff[ph][t0]
                            gw = goff[ph][t0 + tbw] - g0
                            gsl = slice(g0, g0 + gw)
                            if gw == 0:
                                tsl0 = slice(t0 * 128, (t0 + tbw) * 128)
                                for c in range(C):
                                    nc.vector.memset(
                                        ft[c][FP:FP + FN, tsl0]
                                        .bitcast(F32), 0.0)
                                continue
                            gn = gnp.tile([128, NG, NR2], F32, tag="gn")
                            for jj in range(gw):
                                nc.gpsimd.indirect_dma_start(
                                    out=gn[:, jj, :], out_offset=None,
                                    in_=n_full.ap()[:],
                                    in_offset=bass.IndirectOffsetOnAxis(
                                        ap=bdst_t[ph][:, g0 + jj:g0 + jj + 1],
                                        axis=0))
                            Oag = sbp.tile([128, NG, 128], BF16, tag="Oag")
                            nc.vector.tensor_tensor(
                                out=Oag[:, :gw, :],
                                in0=bsrel_t[ph][:, gsl]
                                    .rearrange("a (b c) -> a b c", c=1)
                                    .to_broadcast([128, gw, 128]),
                                in1=iotab_t[:].rearrange("a (b c) -> a b c",
                                                         b=1)
                                    .to_broadcast([128, gw, 128]),
                                op=ALU.is_equal)
                            Oge = sbp.tile([128, NG, 128], BF16, tag="Oge")
                            nc.vector.tensor_tensor(
                                out=Oge[:, :gw, :],
                                in0=iotab_t[:].rearrange("a (b c) -> a b c",
                                                         b=1)
                                    .to_broadcast([128, gw, 128]),
                                in1=bstart_t[ph][:, gsl]
                                    .rearrange("a (b c) -> a b c", c=1)
                                    .to_broadcast([128, gw, 128]),
                                op=ALU.is_ge)
                            Obc = sbp.tile([128, NG, 128], BF16, tag="Obc")
                            nc.vector.tensor_tensor(
                                out=Obc[:, :gw, :],
                                in0=iotab_t[:].rearrange("a (b c) -> a b c",
                                                         b=1)
                                    .to_broadcast([128, gw, 128]),
                                in1=bend_t[ph][:, gsl]
                                    .rearrange("a (b c) -> a b c", c=1)
                                    .to_broadcast([128, gw, 128]),
                                op=ALU.is_lt)
                            nc.vector.tensor_tensor(
                                out=Obc[:, :gw, :], in0=Obc[:, :gw, :],
                                in1=Oge[:, :gw, :], op=ALU.mult)
                            pa8 = bps.tile([128, NG, 6], F32, tag="pa8",
                                           space="PSUM")
                            for ti in range(tbw):
                                t = t0 + ti
                                for jj2 in range(Kt[ph][t]):
                                    jj = goff[ph][t] - g0 + jj2
                                    nc.tensor.matmul(
                                        out=pa8[:, jj, :],
                                        lhsT=Obc[:, jj, :],
                                        rhs=a_sb[:, t, 0:6],
                                        start=True, stop=True)
                            lg = sbp.tile([128, NG, C], F32, tag="lg")
                            nc.vector.tensor_tensor(
                                out=lg[:, :gw, :], in0=pa8[:, :gw, 0:5],
                                in1=gn[:, :gw, CN + p:CN + p
                                       + (C - 1) * P + 1:P],
                                op=ALU.add)
                            mx = sbp.tile([128, NG], F32, tag="mx")
                            nc.vector.tensor_reduce(
                                out=mx[:, :gw], in_=lg[:, :gw, :],
                                axis=mybir.AxisListType.X, op=ALU.max)
                            nc.vector.tensor_tensor(
                                out=lg[:, :gw, :], in0=lg[:, :gw, :],
                                in1=mx[:, :gw].rearrange("a (b c) -> a b c", c=1)
                                    .to_broadcast([128, gw, C]),
                                op=ALU.subtract)
                            ex = sbp.tile([128, NG, C], F32, tag="ex")
                            nc.scalar.activation(ex[:, :gw, :], lg[:, :gw, :],
                                                 EXP)
                            sm = sbp.tile([128, NG], F32, tag="sm")
                            nc.vector.tensor_reduce(
                                out=sm[:, :gw], in_=ex[:, :gw, :],
                                axis=mybir.AxisListType.X, op=ALU.add)
                            nc.vector.reciprocal(out=sm[:, :gw],
                                                 in_=sm[:, :gw])
                            nc.vector.tensor_tensor(
                                out=sm[:, :gw], in0=sm[:, :gw],
                                in1=pa8[:, :gw, 5], op=ALU.mult)
                            nc.vector.tensor_tensor(
                                out=ex[:, :gw, :], in0=ex[:, :gw, :],
                                in1=sm[:, :gw].rearrange("a (b c) -> a b c", c=1)
                                    .to_broadcast([128, gw, C]),
                                op=ALU.mult)
                            msg = sbp.tile([128, NG, CN], BF16, tag="msg")
                            nc.vector.tensor_tensor(
                                out=msg[:, :gw, :].rearrange(
                                    "a b (c f) -> a b c f", f=FN),
                                in0=gn[:, :gw, 0:CN].rearrange(
                                    "a b (c f) -> a b c f", f=FN),
                                in1=ex[:, :gw, :].rearrange(
                                    "a b (c d) -> a b c d", d=1)
                                    .to_broadcast([128, gw, C, FN]),
                                op=ALU.mult)
                            psA = agp.tile([128, TBMAX * 128], F32,
                                           tag="psA", space="PSUM")
                            psB = agp.tile([FN, TBMAX * 128], F32,
                                           tag="psB", space="PSUM")
                            nzw = 0
                            for ti in range(tbw):
                                t = t0 + ti
                                kt = Kt[ph][t]
                                if kt == 0:
                                    break
                                nzw += 1
                                bsl = slice(ti * 128, (ti + 1) * 128)
                                for jj2 in range(kt):
                                    jj = goff[ph][t] - g0 + jj2
                                    nc.tensor.matmul(
                                        out=psA[:, bsl],
                                        lhsT=msg[:, jj, 0:128],
                                        rhs=Oag[:, jj, :],
                                        start=(jj2 == 0),
                                        stop=(jj2 == kt - 1))
                                for jj2 in range(kt):
                                    jj = goff[ph][t] - g0 + jj2
                                    nc.tensor.matmul(
                                        out=psB[:, bsl],
                                        lhsT=msg[:, jj, 128:CN],
                                        rhs=Oag[:, jj, :],
                                        start=(jj2 == 0),
                                        stop=(jj2 == kt - 1))
                            csl2 = slice(t0 * 128, (t0 + nzw) * 128)
                            if nzw > 0:
                                for c in range(4):
                                    nc.vector.tensor_copy(
                                        out=ft[c][FP:FP + FN, csl2],
                                        in_=psA[c * FN:(c + 1) * FN,
                                                0:nzw * 128])
                                nc.vector.tensor_copy(
                                    out=ft[4][FP:FP + FN, csl2],
                                    in_=psB[:, 0:nzw * 128])
                            if nzw < tbw:
                                zsl = slice((t0 + nzw) * 128,
                                            (t0 + tbw) * 128)
                                for c in range(C):
                                    nc.vector.memset(
                                        ft[c][FP:FP + FN, zsl]
                                        .bitcast(F32), 0.0)
                    # ---- stage C: down MLP ----
                    if "c" in stages:
                        for ch0 in range(0, NHP, CHW):
                            cw = min(CHW, NHP - ch0)
                            csl = slice(ch0, ch0 + cw)
                            for c in range(C):
                                hps = scp.tile([FP, CHW], F32, tag="hps",
                                               space="PSUM")
                                nc.tensor.matmul(
                                    out=hps[:, :cw],
                                    lhsT=wd1t_t[p][:, c * FP:(c + 1) * FP],
                                    rhs=ft[c][:, csl], start=True, stop=True)
                                ht = scb.tile([FP, CHW], F32R, tag="ht")
                                nc.scalar.activation(ht[:, :cw], hps[:, :cw],
                                                     TANH, bias=bd1c_t[p][c][:])
                                ops_ = scp.tile([FP, CHW], F32, tag="ops",
                                                space="PSUM")
                                nc.tensor.matmul(
                                    out=ops_[:, :cw], lhsT=wd2t_t[p][c][:],
                                    rhs=ht[:, :cw], start=True, stop=True)
                                ot = scb.tile([FP, CHW], F32, tag="ot")
                                nc.scalar.activation(ot[:, :cw], ops_[:, :cw],
                                                     TANH, bias=bd2c_t[p][c][:])
                                nc.sync.dma_start(
                                    out=out_d[p, h, c, :, csl],
                                    in_=ot[:, :cw])

    nc.compile()
    return nc


_CACHE = {}


def _get_compiled(inputs, cfg):
    in_maps, meta = host_prep(inputs, cfg)
    key = (meta["K_UP"], meta["Kt"], tuple(sorted(cfg.items())))
    if key not in _CACHE:
        _CACHE[key] = build_kernel(meta)
    return _CACHE[key], in_maps, meta


def assemble_output(results, meta):
    cfg = meta["cfg"]
    P, N, C, FP, NC = (cfg[k] for k in ("P", "N", "C", "FP", "NC"))
    NH = meta["NH"]
    pc, phh, pl = meta["perm_core"], meta["perm_h"], meta["perm_loc"]
    arr = np.stack([np.asarray(results[k]["outT"])[:, :, :, :, :NH]
                    for k in range(NC)])      # [NC, P, 2, C, FP, NH]
    out = np.empty((P, N, C, FP), np.float32)
    for p in range(P):
        out[p] = arr[pc[p], p, phh[p], :, :, pl[p]]
    return np.ascontiguousarray(out)


def kernel(**inputs):
    from concourse.bass_utils import run_bass_kernel_spmd
    cfg = CFG_FULL
    nc, in_maps, meta = _get_compiled(inputs, cfg)
    res = run_bass_kernel_spmd(nc, in_maps, list(range(cfg["NC"])))
    return assemble_output(res.results, meta)



# revision 31
# speedup vs baseline: 1.6077x; 1.6077x over previous
"""NexusNet GNN message-passing kernel v3 for 8 Trainium2 NeuronCores.

Sharding:
  - nexus_up + nexus MLP: sharded by nexus node (M/8 contiguous segs/core);
    edges routed to the core owning their dst segment (host index prep).
    x gathered from a bf16 table via 128-row indirect DMAs; aggregation via
    one-hot matmul on PE into PSUM per 128-seg block.
  - n [M,192] f32 rows (160 n + 15 b + pad) AllGathered to every core.
  - down: sharded by planar node (N/8 per core, 2 halves/core/plane), fused
    A+B+C per (plane,half), bf16 compute:
      A: x loaded bf16 feature-major into ft; per-class logits aT computed
         chunk-wise with we1 stationary, transposed per 128-block into the
         node-major a_sb table; invdeg folded in.
      B: per pack of src blocks: ONE batched dma_gather of the pack's edge
         n-rows; softmax weights (no max-subtract; logits are bounded);
         one-hot matmul aggregates messages feature-major into ft.
      C: 2-layer down MLP bf16, classes processed in pairs packed into the
         full 128 PSUM partitions; output transposed on host.
"""

import numpy as np

import concourse.bass as bass
import concourse.bacc as bacc
import concourse.mybir as mybir
import concourse.tile as tile

F32 = mybir.dt.float32
F32R = mybir.dt.float32r
BF16 = mybir.dt.bfloat16
I32 = mybir.dt.int32
I16 = mybir.dt.int16
TANH = mybir.ActivationFunctionType.Tanh
EXP = mybir.ActivationFunctionType.Exp
ALU = mybir.AluOpType

CFG_FULL = dict(P=3, N=100000, M=30000, E=200000, C=5, FP=64, FN=32, NC=8)

GRP = 4               # up-phase seg blocks per nexus-MLP group
CHW = 512             # down-MLP chunk width (4 src blocks)
NR = 192              # n-row floats (160 n + 15 b + 17 pad); 768B %256==0
GMAX = 8              # max 128-idx columns per dma_gather call (ring limit)


def _ceil(a, b):
    return (a + b - 1) // b


def _wrap_idx16(idx, ncols):
    """Flat row-index list -> [128, ncols] int16 wrapped (i%16, i//16),
    replicated across the 8 gpsimd cores."""
    n = len(idx)
    a = np.zeros((16, ncols), np.int16)
    a[np.arange(n) % 16, np.arange(n) // 16] = idx.astype(np.int16)
    return np.tile(a, (8, 1))


def host_prep(inputs, cfg):
    P, N, M, E, C, FP, FN, NC = (cfg[k] for k in
                                 ("P", "N", "M", "E", "C", "FP", "FN", "NC"))
    CF = C * FP
    CN = C * FN
    M_LOC = M // NC
    N_LOC = N // NC
    NH = N_LOC // 2                       # nodes per half
    NHP = _ceil(NH, 128) * 128            # padded half
    NMT = NHP // 128                      # src blocks per half
    NB = _ceil(M_LOC, 128)                # up seg blocks per core

    x = np.ascontiguousarray(np.asarray(inputs["x"], np.float32)
                             .reshape(P, N, CF))
    esrc = np.asarray(inputs["edge_src"])
    edst = np.asarray(inputs["edge_dst"])

    bfnp = mybir.dt.np(BF16)
    xbf = np.ascontiguousarray(x.reshape(P * N, CF)).astype(bfnp)

    # ---------------- UP phase indices ----------------
    per_kp = {}
    max_blk_cnt = 0
    for p in range(P):
        order = np.argsort(edst[p], kind="stable")
        ds, ss = edst[p][order], esrc[p][order]
        bounds = np.searchsorted(ds, np.arange(NC + 1) * M_LOC)
        for k in range(NC):
            sl = slice(bounds[k], bounds[k + 1])
            dsl = (ds[sl] - k * M_LOC).astype(np.int64)
            blk = dsl >> 7
            cnt = np.bincount(blk, minlength=NB)
            max_blk_cnt = max(max_blk_cnt, int(cnt.max(initial=0)))
            per_kp[(k, p)] = (dsl, (ss[sl] + p * N).astype(np.int64), blk, cnt)
    K_UP = max(1, _ceil(max_blk_cnt, 128))
    # per-(p, b) column count: max over cores (SPMD program is shared)
    kb = np.ones((P, NB), np.int64)
    for (k, p), (dsl, sglob, blk, cnt) in per_kp.items():
        kb[p] = np.maximum(kb[p], (cnt + 127) // 128)
    kboff = np.zeros((P, NB + 1), np.int64)
    kboff[:, 1:] = np.cumsum(kb, axis=1)
    NBK = int(kboff[:, -1].max())

    up_src = np.zeros((NC, P, NBK * 128), np.int32)
    up_dr = np.full((NC, P, NBK * 128), -1.0, np.float32)
    for (k, p), (dsl, sglob, blk, cnt) in per_kp.items():
        starts = np.concatenate(([0], np.cumsum(cnt)))[:-1]
        r = np.arange(len(dsl)) - np.repeat(starts, cnt)
        pos = kboff[p][blk] * 128 + r
        up_src[k, p, pos] = sglob
        up_dr[k, p, pos] = dsl - (blk << 7)
    up_src = up_src.reshape(NC, P, NBK, 128).transpose(0, 1, 3, 2).copy()
    up_dr = (up_dr.reshape(NC, P, NBK, 128).transpose(0, 1, 3, 2)
             .astype(bfnp).copy())

    # ---------------- DOWN (stage B) indices ----------------
    # Per-plane striped degree-sort relabel: sorted rank r -> core r%NC,
    # local slot r//NC (-> half, loc).
    perm_core = np.empty((P, N), np.int64)
    perm_h = np.empty((P, N), np.int64)
    perm_loc = np.empty((P, N), np.int64)
    edges = {}
    for p in range(P):
        degp = np.bincount(esrc[p], minlength=N)
        orderp = np.argsort(-degp, kind="stable")
        rank = np.empty(N, np.int64)
        rank[orderp] = np.arange(N)
        loc = rank // NC
        perm_core[p] = rank % NC
        perm_h[p] = loc // NH
        perm_loc[p] = loc % NH
        ec, eh = perm_core[p][esrc[p]], perm_h[p][esrc[p]]
        el, ed = perm_loc[p][esrc[p]], edst[p].astype(np.int64)
        for k in range(NC):
            for h in range(2):
                sel = (ec == k) & (eh == h)
                s_loc = el[sel]
                d_loc = ed[sel]
                o2 = np.argsort(s_loc, kind="stable")
                edges[(k, p, h)] = (s_loc[o2], d_loc[o2])
    # chunk-major n_full layout: AllGather chunk j (CH local rows) lands
    # contiguously at NC*CH*j; nexus id m -> (core m//M_LOC, loc m%M_LOC)
    CH = GRP * 128
    J_AG = _ceil(M_LOC, CH)
    lastch = M_LOC - (J_AG - 1) * CH

    def g2r(m):
        k_ = m // M_LOC
        r_ = m % M_LOC
        j_ = np.minimum(r_ // CH, J_AG - 1)
        base = NC * CH * j_
        chs = np.where(j_ < J_AG - 1, CH, lastch)
        return base + k_ * chs + (r_ - CH * j_)

    # profiles
    Kt = np.zeros((2 * P, NMT), np.int64)
    cnts = {}
    for (k, p, h), (s_loc, d_loc) in edges.items():
        ph = p * 2 + h
        cnt = np.bincount(s_loc >> 7, minlength=NMT)
        cnts[(k, p, h)] = cnt
        Kt[ph] = np.maximum(Kt[ph], (cnt + 127) // 128)
    goff = np.zeros((2 * P, NMT + 1), np.int64)
    goff[:, 1:] = np.cumsum(Kt, axis=1)
    G2 = int(goff[:, -1].max())

    bdst = np.zeros((NC, 2 * P, 128, G2), np.int64)
    bsrel = np.full((NC, 2 * P, 128, G2), -1.0, bfnp)
    bstart = np.zeros((NC, 2 * P, 128, G2), bfnp)
    bend = np.zeros((NC, 2 * P, 128, G2), bfnp)
    degt = np.ones((NC, 2 * P, 128, NMT), np.float32)
    for (k, p, h), (s_loc, d_loc) in edges.items():
        ph = p * 2 + h
        cnt = cnts[(k, p, h)]
        deg = np.bincount(s_loc, minlength=NHP)
        degt[k, ph, :, :] = np.maximum(
            deg.reshape(NMT, 128).T, 1.0).astype(np.float32)
        bb = np.concatenate(([0], np.cumsum(cnt)))
        for t in range(NMT):
            seg_s = s_loc[bb[t]:bb[t + 1]] - 128 * t
            seg_d = d_loc[bb[t]:bb[t + 1]]
            n_t = len(seg_s)
            cum = np.concatenate(
                ([0], np.cumsum(np.bincount(seg_s, minlength=128))))
            for jj in range(int(Kt[ph][t])):
                g = int(goff[ph][t]) + jj
                lo = jj * 128
                nh_ = min(max(n_t - lo, 0), 128)
                if nh_ > 0:
                    bdst[k, ph, :nh_, g] = g2r(seg_d[lo:lo + nh_])
                    bsrel[k, ph, :nh_, g] = seg_s[lo:lo + nh_]
                bstart[k, ph, :, g] = np.clip(cum[:128] - lo, 0, 128)
                bend[k, ph, :, g] = np.clip(cum[1:] - lo, 0, 128)
    # int16 wrapped gather indices: slot (jj*128 + j) at (s%16, s//16), x8
    bidx = np.zeros((NC, 2 * P, 128, G2 * 8), np.int16)
    for k in range(NC):
        for ph in range(2 * P):
            flat = bdst[k, ph].T.reshape(-1)          # [G2*128] slot-major
            bidx[k, ph] = _wrap_idx16(flat, G2 * 8)

    # per-core feature-major bf16 x slices after relabel: [NC, P, 2, CF, NH]
    inv = np.empty((P, NC, 2, NH), np.int64)
    for p in range(P):
        inv[p, perm_core[p], perm_h[p], perm_loc[p]] = np.arange(N)
    xloc = np.empty((NC, P, 2, CF, NH), bfnp)
    for p in range(P):
        for k in range(NC):
            for h in range(2):
                xloc[k, p, h] = x[p][inv[p, k, h]].T.astype(bfnp)

    # ---------------- weights ----------------
    g = lambda n: np.asarray(inputs[n], np.float32)
    Wn1, Wn2, We, Wd1, Wd2 = g("Wn1"), g("Wn2"), g("We"), g("Wd1"), g("Wd2")
    bn1, bn2, be, bd1, bd2 = g("bn1"), g("bn2"), g("be"), g("bd1"), g("bd2")

    wn1t = np.stack([Wn1.transpose(2, 0, 1)[p * FP:(p + 1) * FP]
                     .reshape(FP, C * FN) for p in range(P)]).copy()
    wn2t = Wn2.transpose(2, 0, 1).reshape(FN, C * FN).copy()
    went = We[:, :, 0, FP:]                                   # [P, C, FN]
    wentA = np.zeros((4 * FN, C * P), np.float32)
    for c in range(4):
        wentA[c * FN:(c + 1) * FN, c * P:(c + 1) * P] = went[:, c, :].T
    wentB = np.zeros((FN + 1, C * P), np.float32)
    wentB[:FN, 4 * P:] = went[:, 4, :].T
    wentB[FN, :] = be[:, :, 0].T.reshape(-1)
    bn1c = bn1.reshape(C, FN, 1).copy()
    bn2c = bn2.reshape(C, FN, 1).copy()
    # per-class masked we1: we1m[p, c, :, c'] = We[p,c,0,:FP] iff c'==c.
    # Accumulating the 5 per-class matmuls in PSUM yields a[node, 0:5].
    we1m = np.zeros((P, C, FP, C), np.float32)
    for c in range(C):
        we1m[:, c, :, c] = We[:, c, 0, :FP]
    we1m = we1m.astype(bfnp)
    wd1t = (Wd1.transpose(0, 3, 1, 2).reshape(P, FP + FN, C * FP)
            .astype(bfnp).copy())
    # wd2 duplicated at partition bases 0 and 64 (PE needs lhsT/rhs bases
    # to match; the paired stage-C rhs lives at base 0 or 64).
    wd2t = Wd2.transpose(0, 1, 3, 2).astype(bfnp)             # [P, C, FP, FP]
    wd2b = np.concatenate([wd2t, wd2t], axis=2).copy()        # [P,C,2FP,FP]
    # class-pair packed biases: pairs (0,1), (2,3), (4,)
    bd1p = np.zeros((P, 3, 2 * FP, 1), np.float32)
    bd2p = np.zeros((P, 3, 2 * FP, 1), np.float32)
    for pi, pr in enumerate(((0, 1), (2, 3), (4,))):
        for i, c in enumerate(pr):
            bd1p[:, pi, i * FP:(i + 1) * FP, 0] = bd1[:, c, :]
            bd2p[:, pi, i * FP:(i + 1) * FP, 0] = bd2[:, c, :]
    iota = np.tile(np.arange(128, dtype=np.float32), (128, 1)).copy()
    iotab = iota.astype(bfnp)
    ident = np.eye(128, dtype=np.float32)
    identb = np.eye(128, dtype=np.float32).astype(bfnp)

    meta = dict(cfg=cfg, M_LOC=M_LOC, N_LOC=N_LOC, NH=NH, NHP=NHP,
                NMT=NMT, NB=NB, K_UP=K_UP, NBK=NBK, G2=G2,
                kb=tuple(tuple(int(v) for v in row) for row in kb),
                kboff=tuple(tuple(int(v) for v in row) for row in kboff),
                Kt=tuple(tuple(int(v) for v in row) for row in Kt),
                goff=tuple(tuple(int(v) for v in row) for row in goff),
                perm_core=perm_core, perm_h=perm_h, perm_loc=perm_loc)

    shared = dict(xbf=xbf, wn1t=wn1t, wn2t=wn2t, wentA=wentA, wentB=wentB,
                  bn1c=bn1c, bn2c=bn2c, we1m=we1m, wd1t=wd1t, wd2b=wd2b,
                  bd1p=bd1p, bd2p=bd2p, iota=iota, iotab=iotab,
                  ident=ident, identb=identb)
    in_maps = []
    for k in range(NC):
        m = dict(shared)
        m.update(xloc=xloc[k], up_src=up_src[k], up_dr=up_dr[k],
                 bidx=bidx[k], bsrel=bsrel[k], bstart=bstart[k],
                 bend=bend[k], degt=degt[k])
        in_maps.append(m)
    return in_maps, meta


def build_kernel(meta, stages=("up", "ag", "a", "b", "c")):
    stages = set(stages)
    cfg = meta["cfg"]
    P, N, M, E, C, FP, FN, NC = (cfg[k] for k in
                                 ("P", "N", "M", "E", "C", "FP", "FN", "NC"))
    M_LOC, NH, NHP, NMT = meta["M_LOC"], meta["NH"], meta["NHP"], meta["NMT"]
    NB, K_UP, NBK = meta["NB"], meta["K_UP"], meta["NBK"]
    KB, KBOFF = meta["kb"], meta["kboff"]
    G2 = meta["G2"]
    Kt, goff = meta["Kt"], meta["goff"]
    K_MAX = max(max(row) for row in Kt)
    CF = C * FP
    CN = C * FN
    assert C == 5

    nc = bacc.Bacc("TRN2", num_devices=NC, num_swdge_queues=4)

    def param(name, shape, dt=F32, out=False):
        return nc.declare_dram_parameter(name, list(shape), dt, isOutput=out)

    xbf_d = param("xbf", [P * N, CF], BF16)
    xloc_d = param("xloc", [P, 2, CF, NH], BF16)
    up_src_d = param("up_src", [P, 128, NBK], I32)
    up_dr_d = param("up_dr", [P, 128, NBK], BF16)
    bidx_d = param("bidx", [2 * P, 128, G2 * 8], I16)
    bsrel_d = param("bsrel", [2 * P, 128, G2], BF16)
    bstart_d = param("bstart", [2 * P, 128, G2], BF16)
    bend_d = param("bend", [2 * P, 128, G2], BF16)
    degt_d = param("degt", [2 * P, 128, NMT])
    wn1t_d = param("wn1t", [P, FP, CN])
    wn2t_d = param("wn2t", [FN, CN])
    wentA_d = param("wentA", [4 * FN, C * P])
    wentB_d = param("wentB", [FN + 1, C * P])
    bn1c_d = param("bn1c", [C, FN, 1])
    bn2c_d = param("bn2c", [C, FN, 1])
    we1m_d = param("we1m", [P, C, FP, C], BF16)
    wd1t_d = param("wd1t", [P, FP + FN, C * FP], BF16)
    wd2b_d = param("wd2b", [P, C, 2 * FP, FP], BF16)
    bd1p_d = param("bd1p", [P, 3, 2 * FP, 1])
    bd2p_d = param("bd2p", [P, 3, 2 * FP, 1])
    iota_d = param("iota", [128, 128])
    iotab_d = param("iotab", [128, 128], BF16)
    ident_d = param("ident", [128, 128])
    identb_d = param("identb", [128, 128], BF16)
    out_d = param("outT", [P, 2, C, FP, NHP], out=True)

    n_loc = nc.dram_tensor("n_loc", [M_LOC, NR], F32)
    n_full = nc.dram_tensor("n_full", [NC * M_LOC, NR], F32,
                            addr_space="Shared")
    # dma_gather cannot source from Shared address space and only moves
    # <=2-byte dtypes; mirror AG chunks into a plain DRAM tensor typed as
    # bf16 byte-pairs (same bytes) and gather from that.
    n_flat = nc.dram_tensor("n_flat", [NC * M_LOC, 2 * NR], BF16)

    PAIRS = ((0, 1), (2, 3), (4,))

    with tile.TileContext(nc) as tc:
        with tc.tile_pool(name="const", bufs=1) as cp:
            iota_t = cp.tile([128, 128], F32)
            nc.sync.dma_start(out=iota_t[:], in_=iota_d[:])
            iotab_t = cp.tile([128, 128], BF16)
            nc.sync.dma_start(out=iotab_t[:], in_=iotab_d[:])
            ident_t = cp.tile([128, 128], F32)
            nc.sync.dma_start(out=ident_t[:], in_=ident_d[:])
            identb_t = cp.tile([128, 128], BF16)
            nc.sync.dma_start(out=identb_t[:], in_=identb_d[:])
            wn1t_t = [cp.tile([FP, CN], F32R, name=f"wn1t{p}")
                      for p in range(P)]
            wn2t_t = cp.tile([FN, CN], F32R)
            wentA_t = cp.tile([4 * FN, C * P], F32R)
            wentB_t = cp.tile([FN + 1, C * P], F32R)
            nc.sync.dma_start(out=wn2t_t[:], in_=wn2t_d[:].bitcast(F32R))
            nc.sync.dma_start(out=wentA_t[:], in_=wentA_d[:].bitcast(F32R))
            nc.sync.dma_start(out=wentB_t[:], in_=wentB_d[:].bitcast(F32R))
            bn1c_t = [cp.tile([FN, 1], F32, name=f"bn1c{c}") for c in range(C)]
            bn2c_t = [cp.tile([FN, 1], F32, name=f"bn2c{c}") for c in range(C)]
            we1m_t = [[cp.tile([FP, C], BF16, name=f"we1m{p}_{c}")
                       for c in range(C)] for p in range(P)]
            wd1t_t = [cp.tile([FP + FN, C * FP], BF16, name=f"wd1t{p}")
                      for p in range(P)]
            wd2b_t = [[cp.tile([2 * FP, FP], BF16, name=f"wd2b{p}_{c}")
                       for c in range(C)] for p in range(P)]
            bd1p_t = [[cp.tile([2 * FP, 1], F32, name=f"bd1p{p}_{i}")
                       for i in range(3)] for p in range(P)]
            bd2p_t = [[cp.tile([2 * FP, 1], F32, name=f"bd2p{p}_{i}")
                       for i in range(3)] for p in range(P)]
            for p in range(P):
                nc.sync.dma_start(out=wn1t_t[p][:], in_=wn1t_d[p].bitcast(F32R))
                for c in range(C):
                    nc.sync.dma_start(out=we1m_t[p][c][:], in_=we1m_d[p, c])
                nc.sync.dma_start(out=wd1t_t[p][:], in_=wd1t_d[p])
                for c in range(C):
                    nc.sync.dma_start(out=wd2b_t[p][c][:], in_=wd2b_d[p, c])
                for i in range(3):
                    nc.sync.dma_start(out=bd1p_t[p][i][:], in_=bd1p_d[p, i])
                    nc.sync.dma_start(out=bd2p_t[p][i][:], in_=bd2p_d[p, i])
            for c in range(C):
                nc.sync.dma_start(out=bn1c_t[c][:], in_=bn1c_d[c])
                nc.sync.dma_start(out=bn2c_t[c][:], in_=bn2c_d[c])
            upsrc_t = [cp.tile([128, NBK], I32, name=f"upsrc{p}")
                       for p in range(P)]
            updr_t = [cp.tile([128, NBK], BF16, name=f"updr{p}")
                      for p in range(P)]
            for p in range(P):
                nc.scalar.dma_start(out=upsrc_t[p][:], in_=up_src_d[p])
                nc.scalar.dma_start(out=updr_t[p][:], in_=up_dr_d[p])
            bidx_t, bsrel_t, bstart_t, bend_t, degt_t = [], [], [], [], []
            for ph in range(2 * P):
                bi = cp.tile([128, G2 * 8], I16, name=f"bidx{ph}")
                nc.scalar.dma_start(out=bi[:], in_=bidx_d[ph])
                bidx_t.append(bi)
                s = cp.tile([128, G2], BF16, name=f"bsrel{ph}")
                nc.scalar.dma_start(out=s[:], in_=bsrel_d[ph])
                bsrel_t.append(s)
                s0 = cp.tile([128, G2], BF16, name=f"bstart{ph}")
                nc.scalar.dma_start(out=s0[:], in_=bstart_d[ph])
                bstart_t.append(s0)
                s1 = cp.tile([128, G2], BF16, name=f"bend{ph}")
                nc.scalar.dma_start(out=s1[:], in_=bend_d[ph])
                bend_t.append(s1)
                dg = cp.tile([128, NMT], F32, name=f"degt{ph}")
                nc.scalar.dma_start(out=dg[:], in_=degt_d[ph])
                degt_t.append(dg)
            ones_f = cp.tile([1, GRP * 128], F32)
            nc.vector.memset(ones_f[:], 1.0)
            ones_r = cp.tile([1, GRP * 128], F32R)
            nc.vector.tensor_copy(out=ones_r[:], in_=ones_f[:])

            # ======================= UP PHASE =======================
            n_loc_ap = n_loc.ap()
            with tc.tile_pool(name="up_sb", bufs=3) as up, \
                 tc.tile_pool(name="up_g", bufs=24) as upg, \
                 tc.tile_pool(name="up_sb1", bufs=2) as up1, \
                 tc.tile_pool(name="up_ps", bufs=2, space="PSUM") as upp, \
                 tc.tile_pool(name="up_ps1", bufs=1, space="PSUM") as upp1, \
                 tc.tile_pool(name="mlp_ps", bufs=1, space="PSUM") as mpp:
                uqr = [0]
                for g0 in (range(0, NB, GRP) if "up" in stages else []):
                    gb = list(range(g0, min(g0 + GRP, NB)))
                    GW = len(gb) * 128
                    upX = [[up1.tile([FP, GRP * 128], F32R,
                                     name=f"upX{p}_{c}", tag=f"upX{p}_{c}")
                            for c in range(C)] for p in range(P)]
                    for p in range(P):
                        for bi, b in enumerate(gb):
                            kbb = KB[p][b]
                            kb0 = KBOFF[p][b]
                            O8 = up.tile([128, K_UP, 128], BF16, tag="O8")
                            csl0 = slice(kb0, kb0 + kbb)
                            nc.vector.tensor_tensor(
                                out=O8[:, 0:kbb, :],
                                in0=updr_t[p][:, csl0]
                                    .rearrange("a (b c) -> a b c", c=1)
                                    .to_broadcast([128, kbb, 128]),
                                in1=iotab_t[:].rearrange("a (b c) -> a b c",
                                                         b=1)
                                    .to_broadcast([128, kbb, 128]),
                                op=ALU.is_equal)
                            pu = upp.tile([128, CF], F32, tag="pu",
                                          space="PSUM")
                            for kk in range(kbb):
                                col = kb0 + kk
                                G = upg.tile([128, CF], BF16, tag="G")
                                gi = nc.gpsimd.indirect_dma_start(
                                    out=G[:], out_offset=None,
                                    in_=xbf_d[:],
                                    in_offset=bass.IndirectOffsetOnAxis(
                                        ap=upsrc_t[p][:, col:col + 1], axis=0))
                                qi = uqr[0] % 4
                                uqr[0] += 1
                                if qi:
                                    gi.ins.queue = f"qPoolDynamic{qi}"
                                nc.tensor.matmul(out=pu[:],
                                                 lhsT=O8[:, kk, :],
                                                 rhs=G[:], start=(kk == 0),
                                                 stop=(kk == kbb - 1))
                            stg = up.tile([128, CF], F32, tag="stg")
                            nc.scalar.copy(out=stg[:], in_=pu[:])
                            csl = slice(bi * 128, (bi + 1) * 128)
                            for ti in range(3):
                                w = min(128, CF - ti * 128)
                                pt = upp1.tile([128, 128], F32, tag="ptr",
                                               space="PSUM")
                                nc.tensor.transpose(
                                    out=pt[:w, :],
                                    in_=stg[:, ti * 128:ti * 128 + w],
                                    identity=ident_t[:])
                                nc.vector.tensor_copy(
                                    out=upX[p][2 * ti][:, csl],
                                    in_=pt[0:FP, :])
                                if 2 * ti + 1 < C:
                                    nc.vector.tensor_copy(
                                        out=upX[p][2 * ti + 1][:, csl],
                                        in_=pt[FP:2 * FP, :])
                    # ---- nexus MLP over this group ----
                    n1c = [up.tile([FN, GRP * 128], F32R, name=f"n1c{c}",
                                   tag=f"n1c{c}") for c in range(C)]
                    for c in range(C):
                        pn1 = mpp.tile([FN, GRP * 128], F32, tag="pn1",
                                       space="PSUM", bufs=2)
                        for p in range(P):
                            nc.tensor.matmul(
                                out=pn1[:, :GW],
                                lhsT=wn1t_t[p][:, c * FN:(c + 1) * FN],
                                rhs=upX[p][c][:, :GW],
                                start=(p == 0), stop=(p == P - 1))
                        nc.scalar.activation(n1c[c][:, :GW], pn1[:, :GW],
                                             TANH, bias=bn1c_t[c][:])
                    n2s = up.tile([4 * FN, GRP * 128], F32R, tag="n2s")
                    nbt = up.tile([FN + 1, GRP * 128], F32R, tag="nbt")
                    nc.vector.tensor_copy(out=nbt[FN:FN + 1, :],
                                          in_=ones_r[:])
                    for c in range(C):
                        pn2 = mpp.tile([FN, GRP * 128], F32, tag="pn2",
                                       space="PSUM", bufs=2)
                        nc.tensor.matmul(
                            out=pn2[:, :GW],
                            lhsT=wn2t_t[:, c * FN:(c + 1) * FN],
                            rhs=n1c[c][:, :GW], start=True, stop=True)
                        dst = (n2s[c * FN:(c + 1) * FN, :GW] if c < 4
                               else nbt[0:FN, :GW])
                        nc.scalar.activation(dst, pn2[:, :GW],
                                             TANH, bias=bn2c_t[c][:])
                    pbv = mpp.tile([C * P, GRP * 128], F32, tag="misc",
                                   space="PSUM", bufs=1)
                    nc.tensor.matmul(out=pbv[:, :GW], lhsT=wentA_t[:],
                                     rhs=n2s[:, :GW], start=True, stop=False)
                    nc.tensor.matmul(out=pbv[:, :GW], lhsT=wentB_t[:],
                                     rhs=nbt[:, :GW], start=False, stop=True)
                    bt = up.tile([C * P, GRP * 128], F32, tag="bt")
                    nc.scalar.copy(out=bt[:, :GW], in_=pbv[:, :GW])
                    # assemble + store n rows per block
                    for bi, b in enumerate(gb):
                        rows = min(128, M_LOC - b * 128)
                        sl = slice(bi * 128, bi * 128 + 128)
                        tp = mpp.tile([128, CN + C * P], F32,
                                      tag="misc", space="PSUM", bufs=1)
                        nc.tensor.transpose(
                            out=tp[:, 0:4 * FN],
                            in_=n2s[:, sl].bitcast(F32),
                            identity=ident_t[:])
                        nc.tensor.transpose(
                            out=tp[:, 4 * FN:CN],
                            in_=nbt[0:FN, sl].bitcast(F32),
                            identity=ident_t[:FN, :FN])
                        nc.tensor.transpose(
                            out=tp[:, CN:CN + C * P],
                            in_=bt[:, sl],
                            identity=ident_t[:C * P, :C * P])
                        nrow = up.tile([128, NR], F32, tag="nrow")
                        nc.scalar.copy(out=nrow[:, 0:CN + C * P], in_=tp[:])
                        nc.vector.memset(nrow[:, CN + C * P:], 0.0)
                        nc.sync.dma_start(
                            out=n_loc_ap[b * 128:b * 128 + rows, :],
                            in_=nrow[:rows, :])
                    if "ag" in stages:
                        CH = GRP * 128
                        jch = g0 // GRP
                        lo = jch * CH
                        hi = min(lo + CH, M_LOC)
                        if hi > lo:
                            base = NC * CH * jch
                            nrows = NC * (hi - lo)
                            nfv = n_full.ap()[base:base + nrows, :]
                            nc.gpsimd.collective_compute(
                                "AllGather", ALU.bypass,
                                replica_groups=[list(range(NC))],
                                ins=[n_loc_ap[lo:hi, :].opt()],
                                outs=[nfv.opt()])
                            nc.sync.dma_start(
                                out=n_flat.ap()[base:base + nrows, :],
                                in_=n_full.ap()[base:base + nrows, :]
                                .bitcast(BF16))

            # ================= AllGather n (ablation fallback) ==========
            if "ag" in stages and "up" not in stages:
                CH = GRP * 128
                J_AG = _ceil(M_LOC, CH)
                for jch in range(J_AG):
                    lo = jch * CH
                    hi = min(lo + CH, M_LOC)
                    base = NC * CH * jch
                    nrows = NC * (hi - lo)
                    nc.gpsimd.collective_compute(
                        "AllGather", ALU.bypass,
                        replica_groups=[list(range(NC))],
                        ins=[n_loc.ap()[lo:hi, :].opt()],
                        outs=[n_full.ap()[base:base + nrows, :].opt()])
                    nc.sync.dma_start(
                        out=n_flat.ap()[base:base + nrows, :],
                        in_=n_full.ap()[base:base + nrows, :].bitcast(BF16))

            # ============ FUSED A+B+C per (plane, half) ============
            NG = max(10, K_MAX)
            TBMAX = 4
            qrr = [0]
            packs = {}                    # ph -> list of (t0, tbw)
            for ph0 in range(2 * P):
                lst = []
                t = 0
                while t < NMT:
                    tw, gsum = 0, 0
                    while (t + tw < NMT and tw < TBMAX
                           and (tw == 0
                                or gsum + Kt[ph0][t + tw] <= NG)):
                        gsum += Kt[ph0][t + tw]
                        tw += 1
                    lst.append((t, tw))
                    t += tw
                packs[ph0] = lst
            with tc.tile_pool(name="ft_sb", bufs=1) as ftp, \
                 tc.tile_pool(name="ab_sb", bufs=2) as ab, \
                 tc.tile_pool(name="gn_sb", bufs=5) as gnp, \
                 tc.tile_pool(name="b_sb", bufs=2) as sbp, \
                 tc.tile_pool(name="b_ps", bufs=1, space="PSUM") as bps, \
                 tc.tile_pool(name="agg_ps", bufs=1, space="PSUM") as agp, \
                 tc.tile_pool(name="c_sb", bufs=2) as scb, \
                 tc.tile_pool(name="c_ps", bufs=2, space="PSUM") as scp:
                for ph in range(2 * P):
                    p, h = ph // 2, ph % 2
                    # ---- stage A: load x into ft (bf16), aT, a_sb ----
                    ft = [ftp.tile([FP + FN, NHP], BF16, name=f"ft{c}",
                                   tag=f"ft{c}") for c in range(C)]
                    a_sb = ab.tile([128, NMT, 8], BF16, tag="a_sb")
                    if "a" in stages:
                        for c in range(C):
                            if NHP > NH:
                                nc.vector.memset(ft[c][0:FP, NH:], 0.0)
                            nc.sync.dma_start(
                                out=ft[c][0:FP, :NH],
                                in_=xloc_d[p, h, c * FP:(c + 1) * FP, :])
                        with nc.allow_low_precision(reason="invdeg bf16"):
                            nc.vector.reciprocal(
                                out=a_sb[:, :, 5:6],
                                in_=degt_t[ph][:]
                                .rearrange("a (b c) -> a b c", c=1))
                        for t0 in range(0, NMT, 8):
                            tw = min(8, NMT - t0)
                            pa = bps.tile([128, NG, 6], F32, tag="pa8",
                                          space="PSUM")
                            for ti in range(tw):
                                t = t0 + ti
                                tsl = slice(t * 128, (t + 1) * 128)
                                for c in range(C):
                                    nc.tensor.matmul(
                                        out=pa[:, ti, 0:C],
                                        lhsT=ft[c][0:FP, tsl],
                                        rhs=we1m_t[p][c][:],
                                        start=(c == 0), stop=(c == C - 1))
                            nc.vector.tensor_copy(
                                out=a_sb[:, t0:t0 + tw, 0:5],
                                in_=pa[:, 0:tw, 0:C])
                    # ---- stage B ----
                    if "b" in stages:
                        for (t0, tbw) in packs[ph]:
                            g0 = goff[ph][t0]
                            gw = goff[ph][t0 + tbw] - g0
                            gsl = slice(g0, g0 + gw)
                            if gw == 0:
                                tsl0 = slice(t0 * 128, (t0 + tbw) * 128)
                                for c in range(C):
                                    nc.vector.memset(
                                        ft[c][FP:FP + FN, tsl0], 0.0)
                                continue
                            gn = gnp.tile([128, NG, NR], F32, tag="gn")
                            gnb = gn[:].bitcast(BF16)
                            for go in range(0, gw, GMAX):
                                gww = min(GMAX, gw - go)
                                nc.gpsimd.dma_gather(
                                    out_ap=gnb[:, go:go + gww, :],
                                    in_ap=n_flat.ap()[:],
                                    idxs_ap=bidx_t[ph][
                                        :, (g0 + go) * 8:(g0 + go + gww) * 8],
                                    num_idxs=gww * 128,
                                    num_idxs_reg=gww * 128,
                                    elem_size=2 * NR,
                                    single_packet=True,
                                    queue_num=qrr[0] % 4)
                                qrr[0] += 1
                            Oag = sbp.tile([128, NG, 128], BF16, tag="Oag")
                            nc.vector.tensor_tensor(
                                out=Oag[:, :gw, :],
                                in0=bsrel_t[ph][:, gsl]
                                    .rearrange("a (b c) -> a b c", c=1)
                                    .to_broadcast([128, gw, 128]),
                                in1=iotab_t[:].rearrange("a (b c) -> a b c",
                                                         b=1)
                                    .to_broadcast([128, gw, 128]),
                                op=ALU.is_equal)
                            Oge = sbp.tile([128, NG, 128], BF16, tag="Oge")
                            nc.vector.tensor_tensor(
                                out=Oge[:, :gw, :],
                                in0=iotab_t[:].rearrange("a (b c) -> a b c",
                                                         b=1)
                                    .to_broadcast([128, gw, 128]),
                                in1=bstart_t[ph][:, gsl]
                                    .rearrange("a (b c) -> a b c", c=1)
                                    .to_broadcast([128, gw, 128]),
                                op=ALU.is_ge)
                            Obc = sbp.tile([128, NG, 128], BF16, tag="Obc")
                            nc.vector.tensor_tensor(
                                out=Obc[:, :gw, :],
                                in0=iotab_t[:].rearrange("a (b c) -> a b c",
                                                         b=1)
                                    .to_broadcast([128, gw, 128]),
                                in1=bend_t[ph][:, gsl]
                                    .rearrange("a (b c) -> a b c", c=1)
                                    .to_broadcast([128, gw, 128]),
                                op=ALU.is_lt)
                            nc.vector.tensor_tensor(
                                out=Obc[:, :gw, :], in0=Obc[:, :gw, :],
                                in1=Oge[:, :gw, :], op=ALU.mult)
                            pa8 = bps.tile([128, NG, 6], F32, tag="pa8",
                                           space="PSUM")
                            for ti in range(tbw):
                                t = t0 + ti
                                for jj2 in range(Kt[ph][t]):
                                    jj = goff[ph][t] - g0 + jj2
                                    nc.tensor.matmul(
                                        out=pa8[:, jj, :],
                                        lhsT=Obc[:, jj, :],
                                        rhs=a_sb[:, t, 0:6],
                                        start=True, stop=True)
                            lg = sbp.tile([128, NG, C], F32, tag="lg")
                            nc.vector.tensor_tensor(
                                out=lg[:, :gw, :], in0=pa8[:, :gw, 0:5],
                                in1=gn[:, :gw, CN + p:CN + p
                                       + (C - 1) * P + 1:P],
                                op=ALU.add)
                            ex = sbp.tile([128, NG, C], F32, tag="ex")
                            nc.scalar.activation(ex[:, :gw, :], lg[:, :gw, :],
                                                 EXP)
                            sm = sbp.tile([128, NG], F32, tag="sm")
                            nc.vector.tensor_reduce(
                                out=sm[:, :gw], in_=ex[:, :gw, :],
                                axis=mybir.AxisListType.X, op=ALU.add)
                            nc.vector.reciprocal(out=sm[:, :gw],
                                                 in_=sm[:, :gw])
                            nc.vector.tensor_tensor(
                                out=sm[:, :gw], in0=sm[:, :gw],
                                in1=pa8[:, :gw, 5], op=ALU.mult)
                            nc.vector.tensor_tensor(
                                out=ex[:, :gw, :], in0=ex[:, :gw, :],
                                in1=sm[:, :gw].rearrange("a (b c) -> a b c",
                                                         c=1)
                                    .to_broadcast([128, gw, C]),
                                op=ALU.mult)
                            msg = sbp.tile([128, NG, CN], BF16, tag="msg")
                            nc.vector.tensor_tensor(
                                out=msg[:, :gw, :].rearrange(
                                    "a b (c f) -> a b c f", f=FN),
                                in0=gn[:, :gw, 0:CN].rearrange(
                                    "a b (c f) -> a b c f", f=FN),
                                in1=ex[:, :gw, :].rearrange(
                                    "a b (c d) -> a b c d", d=1)
                                    .to_broadcast([128, gw, C, FN]),
                                op=ALU.mult)
                            psA = agp.tile([128, TBMAX * 128], F32,
                                           tag="psA", space="PSUM")
                            psB = agp.tile([FN, TBMAX * 128], F32,
                                           tag="psB", space="PSUM")
                            nzw = 0
                            for ti in range(tbw):
                                t = t0 + ti
                                kt = Kt[ph][t]
                                if kt == 0:
                                    break
                                nzw += 1
                                bsl = slice(ti * 128, (ti + 1) * 128)
                                for jj2 in range(kt):
                                    jj = goff[ph][t] - g0 + jj2
                                    nc.tensor.matmul(
                                        out=psA[:, bsl],
                                        lhsT=msg[:, jj, 0:128],
                                        rhs=Oag[:, jj, :],
                                        start=(jj2 == 0),
                                        stop=(jj2 == kt - 1))
                                for jj2 in range(kt):
                                    jj = goff[ph][t] - g0 + jj2
                                    nc.tensor.matmul(
                                        out=psB[:, bsl],
                                        lhsT=msg[:, jj, 128:CN],
                                        rhs=Oag[:, jj, :],
                                        start=(jj2 == 0),
                                        stop=(jj2 == kt - 1))
                            csl2 = slice(t0 * 128, (t0 + nzw) * 128)
                            if nzw > 0:
                                for c in range(4):
                                    nc.scalar.copy(
                                        out=ft[c][FP:FP + FN, csl2],
                                        in_=psA[c * FN:(c + 1) * FN,
                                                0:nzw * 128])
                                nc.scalar.copy(
                                    out=ft[4][FP:FP + FN, csl2],
                                    in_=psB[:, 0:nzw * 128])
                            if nzw < tbw:
                                zsl = slice((t0 + nzw) * 128,
                                            (t0 + tbw) * 128)
                                for c in range(C):
                                    nc.vector.memset(
                                        ft[c][FP:FP + FN, zsl], 0.0)
                    # ---- stage C: down MLP (bf16, class pairs) ----
                    if "c" in stages:
                        for ch0 in range(0, NHP, CHW):
                            cw = min(CHW, NHP - ch0)
                            csl = slice(ch0, ch0 + cw)
                            for pi, pr in enumerate(PAIRS):
                                rows = len(pr) * FP
                                hps = scp.tile([2 * FP, CHW], F32, tag="hps",
                                               space="PSUM")
                                for i, c in enumerate(pr):
                                    nc.tensor.matmul(
                                        out=hps[i * FP:(i + 1) * FP, :cw],
                                        lhsT=wd1t_t[p][:, c * FP:(c + 1) * FP],
                                        rhs=ft[c][:, csl],
                                        start=True, stop=True)
                                ht = scb.tile([2 * FP, CHW], BF16, tag="ht")
                                nc.scalar.activation(
                                    ht[0:rows, :cw], hps[0:rows, :cw],
                                    TANH, bias=bd1p_t[p][pi][0:rows])
                                ops_ = scp.tile([2 * FP, CHW], F32, tag="ops",
                                                space="PSUM")
                                for i, c in enumerate(pr):
                                    nc.tensor.matmul(
                                        out=ops_[i * FP:(i + 1) * FP, :cw],
                                        lhsT=wd2b_t[p][c][
                                            i * FP:(i + 1) * FP, :],
                                        rhs=ht[i * FP:(i + 1) * FP, :cw],
                                        start=True, stop=True)
                                ot = scb.tile([2 * FP, CHW], F32, tag="ot")
                                nc.scalar.activation(
                                    ot[0:rows, :cw], ops_[0:rows, :cw],
                                    TANH, bias=bd2p_t[p][pi][0:rows])
                                for i, c in enumerate(pr):
                                    nc.sync.dma_start(
                                        out=out_d[p, h, c, :, csl],
                                        in_=ot[i * FP:(i + 1) * FP, :cw])

    nc.compile()
    return nc


_CACHE = {}


def _get_compiled(inputs, cfg):
    in_maps, meta = host_prep(inputs, cfg)
    key = (meta["K_UP"], meta["Kt"], tuple(sorted(cfg.items())))
    if key not in _CACHE:
        _CACHE[key] = build_kernel(meta)
    return _CACHE[key], in_maps, meta


def assemble_output(results, meta):
    cfg = meta["cfg"]
    P, N, C, FP, NC = (cfg[k] for k in ("P", "N", "C", "FP", "NC"))
    NH = meta["NH"]
    pc, phh, pl = meta["perm_core"], meta["perm_h"], meta["perm_loc"]
    arr = np.stack([np.asarray(results[k]["outT"])[:, :, :, :, :NH]
                    for k in range(NC)])      # [NC, P, 2, C, FP, NH]
    out = np.empty((P, N, C, FP), np.float32)
    for p in range(P):
        out[p] = arr[pc[p], p, phh[p], :, :, pl[p]]
    return np.ascontiguousarray(out)


def kernel(**inputs):
    from concourse.bass_utils import run_bass_kernel_spmd
    cfg = CFG_FULL
    nc, in_maps, meta = _get_compiled(inputs, cfg)
    res = run_bass_kernel_spmd(nc, in_maps, list(range(cfg["NC"])))
    return assemble_output(res.results, meta)


# revision 36
# speedup vs baseline: 1.6122x; 1.0028x over previous
"""NexusNet GNN message-passing kernel v3 for 8 Trainium2 NeuronCores.

Sharding:
  - nexus_up + nexus MLP: sharded by nexus node (M/8 contiguous segs/core);
    edges routed to the core owning their dst segment (host index prep).
    x gathered from a bf16 table via 128-row indirect DMAs; aggregation via
    one-hot matmul on PE into PSUM per 128-seg block.
  - n [M,192] f32 rows (160 n + 15 b + pad) AllGathered to every core.
  - down: sharded by planar node (N/8 per core, 2 halves/core/plane), fused
    A+B+C per (plane,half), bf16 compute:
      A: x loaded bf16 feature-major into ft; per-node per-class logits via
         PSUM-accumulated matmuls against class-masked we1 (node-major a_sb
         directly); invdeg folded in.
      B: per pack of src blocks: batched dma_gather (int16 idx, round-robin
         SWDGE queues) of the pack's edge n-rows from a non-shared mirror of
         the AllGathered table; softmax weights (no max-subtract; logits are
         bounded); one-hot matmul aggregates messages feature-major into ft.
      C: 2-layer down MLP bf16, classes processed in pairs packed into the
         full 128 PSUM partitions; output transposed on host.

  SWDGE notes (hard-won): descriptor generation costs ~8.5ns/row on the Q7
  regardless of batching; dma_gather crashes on >1024 idx with
  single_packet=True (64-desc packet cap), on fp32 sources, and is limited
  to int16 row indices; collectives must write Shared space but dma_gather
  must read non-Shared, hence the n_flat mirror.
"""

import numpy as np

import concourse.bass as bass
import concourse.bacc as bacc
import concourse.mybir as mybir
import concourse.tile as tile

F32 = mybir.dt.float32
F32R = mybir.dt.float32r
BF16 = mybir.dt.bfloat16
I32 = mybir.dt.int32
I16 = mybir.dt.int16
TANH = mybir.ActivationFunctionType.Tanh
EXP = mybir.ActivationFunctionType.Exp
ALU = mybir.AluOpType

CFG_FULL = dict(P=3, N=100000, M=30000, E=200000, C=5, FP=64, FN=32, NC=8)

GRP = 4               # up-phase seg blocks per nexus-MLP group
CHW = 512             # down-MLP chunk width (4 src blocks)
NR = 192              # n-row floats (160 n + 15 b + 17 pad); 768B %256==0
GMAX = 8              # max 128-idx columns per dma_gather call (ring limit)


def _ceil(a, b):
    return (a + b - 1) // b


def _wrap_idx16(idx, ncols):
    """Flat row-index list -> [128, ncols] int16 wrapped (i%16, i//16),
    replicated across the 8 gpsimd cores."""
    n = len(idx)
    a = np.zeros((16, ncols), np.int16)
    a[np.arange(n) % 16, np.arange(n) // 16] = idx.astype(np.int16)
    return np.tile(a, (8, 1))


def host_prep(inputs, cfg):
    P, N, M, E, C, FP, FN, NC = (cfg[k] for k in
                                 ("P", "N", "M", "E", "C", "FP", "FN", "NC"))
    CF = C * FP
    CN = C * FN
    M_LOC = M // NC
    N_LOC = N // NC
    NH = N_LOC // 2                       # nodes per half
    NHP = _ceil(NH, 128) * 128            # padded half
    NMT = NHP // 128                      # src blocks per half
    NB = _ceil(M_LOC, 128)                # up seg blocks per core

    x = np.ascontiguousarray(np.asarray(inputs["x"], np.float32)
                             .reshape(P, N, CF))
    esrc = np.asarray(inputs["edge_src"])
    edst = np.asarray(inputs["edge_dst"])

    bfnp = mybir.dt.np(BF16)
    xbf = np.ascontiguousarray(x.reshape(P * N, CF)).astype(bfnp)

    # ---------------- UP phase indices ----------------
    per_kp = {}
    max_blk_cnt = 0
    for p in range(P):
        order = np.argsort(edst[p], kind="stable")
        ds, ss = edst[p][order], esrc[p][order]
        bounds = np.searchsorted(ds, np.arange(NC + 1) * M_LOC)
        for k in range(NC):
            sl = slice(bounds[k], bounds[k + 1])
            dsl = (ds[sl] - k * M_LOC).astype(np.int64)
            blk = dsl >> 7
            cnt = np.bincount(blk, minlength=NB)
            max_blk_cnt = max(max_blk_cnt, int(cnt.max(initial=0)))
            per_kp[(k, p)] = (dsl, (ss[sl] + p * N).astype(np.int64), blk, cnt)
    K_UP = max(1, _ceil(max_blk_cnt, 128))
    # per-(p, b) column count: max over cores (SPMD program is shared)
    kb = np.ones((P, NB), np.int64)
    for (k, p), (dsl, sglob, blk, cnt) in per_kp.items():
        kb[p] = np.maximum(kb[p], (cnt + 127) // 128)
    kboff = np.zeros((P, NB + 1), np.int64)
    kboff[:, 1:] = np.cumsum(kb, axis=1)
    NBK = int(kboff[:, -1].max())

    up_src = np.zeros((NC, P, NBK * 128), np.int32)
    up_dr = np.full((NC, P, NBK * 128), -1.0, np.float32)
    for (k, p), (dsl, sglob, blk, cnt) in per_kp.items():
        starts = np.concatenate(([0], np.cumsum(cnt)))[:-1]
        r = np.arange(len(dsl)) - np.repeat(starts, cnt)
        pos = kboff[p][blk] * 128 + r
        up_src[k, p, pos] = sglob
        up_dr[k, p, pos] = dsl - (blk << 7)
    up_src = up_src.reshape(NC, P, NBK, 128).transpose(0, 1, 3, 2).copy()
    up_dr = (up_dr.reshape(NC, P, NBK, 128).transpose(0, 1, 3, 2)
             .astype(bfnp).copy())

    # ---------------- DOWN (stage B) indices ----------------
    # Per-plane striped degree-sort relabel: sorted rank r -> core r%NC,
    # local slot r//NC (-> half, loc).
    perm_core = np.empty((P, N), np.int64)
    perm_h = np.empty((P, N), np.int64)
    perm_loc = np.empty((P, N), np.int64)
    edges = {}
    for p in range(P):
        degp = np.bincount(esrc[p], minlength=N)
        orderp = np.argsort(-degp, kind="stable")
        rank = np.empty(N, np.int64)
        rank[orderp] = np.arange(N)
        loc = rank // NC
        perm_core[p] = rank % NC
        perm_h[p] = loc // NH
        perm_loc[p] = loc % NH
        ec, eh = perm_core[p][esrc[p]], perm_h[p][esrc[p]]
        el, ed = perm_loc[p][esrc[p]], edst[p].astype(np.int64)
        for k in range(NC):
            for h in range(2):
                sel = (ec == k) & (eh == h)
                s_loc = el[sel]
                d_loc = ed[sel]
                o2 = np.argsort(s_loc, kind="stable")
                edges[(k, p, h)] = (s_loc[o2], d_loc[o2])
    # chunk-major n_full layout: AllGather chunk j (CH local rows) lands
    # contiguously at NC*CH*j; nexus id m -> (core m//M_LOC, loc m%M_LOC)
    CH = GRP * 128
    J_AG = _ceil(M_LOC, CH)
    lastch = M_LOC - (J_AG - 1) * CH

    def g2r(m):
        k_ = m // M_LOC
        r_ = m % M_LOC
        j_ = np.minimum(r_ // CH, J_AG - 1)
        base = NC * CH * j_
        chs = np.where(j_ < J_AG - 1, CH, lastch)
        return base + k_ * chs + (r_ - CH * j_)

    # profiles
    Kt = np.zeros((2 * P, NMT), np.int64)
    cnts = {}
    for (k, p, h), (s_loc, d_loc) in edges.items():
        ph = p * 2 + h
        cnt = np.bincount(s_loc >> 7, minlength=NMT)
        cnts[(k, p, h)] = cnt
        Kt[ph] = np.maximum(Kt[ph], (cnt + 127) // 128)
    goff = np.zeros((2 * P, NMT + 1), np.int64)
    goff[:, 1:] = np.cumsum(Kt, axis=1)
    G2 = int(goff[:, -1].max())

    bdst = np.zeros((NC, 2 * P, 128, G2), np.int64)
    bsrel = np.full((NC, 2 * P, 128, G2), -1.0, bfnp)
    bstart = np.zeros((NC, 2 * P, 128, G2), bfnp)
    bend = np.zeros((NC, 2 * P, 128, G2), bfnp)
    degt = np.ones((NC, 2 * P, 128, NMT), np.float32)
    for (k, p, h), (s_loc, d_loc) in edges.items():
        ph = p * 2 + h
        cnt = cnts[(k, p, h)]
        deg = np.bincount(s_loc, minlength=NHP)
        degt[k, ph, :, :] = np.maximum(
            deg.reshape(NMT, 128).T, 1.0).astype(np.float32)
        bb = np.concatenate(([0], np.cumsum(cnt)))
        for t in range(NMT):
            seg_s = s_loc[bb[t]:bb[t + 1]] - 128 * t
            seg_d = d_loc[bb[t]:bb[t + 1]]
            n_t = len(seg_s)
            cum = np.concatenate(
                ([0], np.cumsum(np.bincount(seg_s, minlength=128))))
            for jj in range(int(Kt[ph][t])):
                g = int(goff[ph][t]) + jj
                lo = jj * 128
                nh_ = min(max(n_t - lo, 0), 128)
                if nh_ > 0:
                    bdst[k, ph, :nh_, g] = g2r(seg_d[lo:lo + nh_])
                    bsrel[k, ph, :nh_, g] = seg_s[lo:lo + nh_]
                bstart[k, ph, :, g] = np.clip(cum[:128] - lo, 0, 128)
                bend[k, ph, :, g] = np.clip(cum[1:] - lo, 0, 128)
    # int16 wrapped gather indices: slot (jj*128 + j) at (s%16, s//16), x8
    bidx = np.zeros((NC, 2 * P, 128, G2 * 8), np.int16)
    for k in range(NC):
        for ph in range(2 * P):
            flat = bdst[k, ph].T.reshape(-1)          # [G2*128] slot-major
            bidx[k, ph] = _wrap_idx16(flat, G2 * 8)

    # per-core feature-major bf16 x slices after relabel: [NC, P, 2, CF, NH]
    inv = np.empty((P, NC, 2, NH), np.int64)
    for p in range(P):
        inv[p, perm_core[p], perm_h[p], perm_loc[p]] = np.arange(N)
    xloc = np.empty((NC, P, 2, CF, NH), bfnp)
    for p in range(P):
        for k in range(NC):
            for h in range(2):
                xloc[k, p, h] = x[p][inv[p, k, h]].T.astype(bfnp)

    # ---------------- weights ----------------
    g = lambda n: np.asarray(inputs[n], np.float32)
    Wn1, Wn2, We, Wd1, Wd2 = g("Wn1"), g("Wn2"), g("We"), g("Wd1"), g("Wd2")
    bn1, bn2, be, bd1, bd2 = g("bn1"), g("bn2"), g("be"), g("bd1"), g("bd2")

    wn1t = np.stack([Wn1.transpose(2, 0, 1)[p * FP:(p + 1) * FP]
                     .reshape(FP, C * FN) for p in range(P)]).copy()
    wn2t = Wn2.transpose(2, 0, 1).reshape(FN, C * FN).copy()
    went = We[:, :, 0, FP:]                                   # [P, C, FN]
    wentA = np.zeros((4 * FN, C * P), np.float32)
    for c in range(4):
        wentA[c * FN:(c + 1) * FN, c * P:(c + 1) * P] = went[:, c, :].T
    wentB = np.zeros((FN + 1, C * P), np.float32)
    wentB[:FN, 4 * P:] = went[:, 4, :].T
    wentB[FN, :] = be[:, :, 0].T.reshape(-1)
    bn1c = bn1.reshape(C, FN, 1).copy()
    bn2c = bn2.reshape(C, FN, 1).copy()
    # per-class masked we1: we1m[p, c, :, c'] = We[p,c,0,:FP] iff c'==c.
    # Accumulating the 5 per-class matmuls in PSUM yields a[node, 0:5].
    we1m = np.zeros((P, C, FP, C), np.float32)
    for c in range(C):
        we1m[:, c, :, c] = We[:, c, 0, :FP]
    we1m = we1m.astype(bfnp)
    wd1t = (Wd1.transpose(0, 3, 1, 2).reshape(P, FP + FN, C * FP)
            .astype(bfnp).copy())
    # wd2 duplicated at partition bases 0 and 64 (PE needs lhsT/rhs bases
    # to match; the paired stage-C rhs lives at base 0 or 64).
    wd2t = Wd2.transpose(0, 1, 3, 2).astype(bfnp)             # [P, C, FP, FP]
    wd2b = np.concatenate([wd2t, wd2t], axis=2).copy()        # [P,C,2FP,FP]
    # class-pair packed biases: pairs (0,1), (2,3), (4,)
    bd1p = np.zeros((P, 3, 2 * FP, 1), np.float32)
    bd2p = np.zeros((P, 3, 2 * FP, 1), np.float32)
    for pi, pr in enumerate(((0, 1), (2, 3), (4,))):
        for i, c in enumerate(pr):
            bd1p[:, pi, i * FP:(i + 1) * FP, 0] = bd1[:, c, :]
            bd2p[:, pi, i * FP:(i + 1) * FP, 0] = bd2[:, c, :]
    iota = np.tile(np.arange(128, dtype=np.float32), (128, 1)).copy()
    iotab = iota.astype(bfnp)
    ident = np.eye(128, dtype=np.float32)
    identb = np.eye(128, dtype=np.float32).astype(bfnp)

    meta = dict(cfg=cfg, M_LOC=M_LOC, N_LOC=N_LOC, NH=NH, NHP=NHP,
                NMT=NMT, NB=NB, K_UP=K_UP, NBK=NBK, G2=G2,
                kb=tuple(tuple(int(v) for v in row) for row in kb),
                kboff=tuple(tuple(int(v) for v in row) for row in kboff),
                Kt=tuple(tuple(int(v) for v in row) for row in Kt),
                goff=tuple(tuple(int(v) for v in row) for row in goff),
                perm_core=perm_core, perm_h=perm_h, perm_loc=perm_loc)

    shared = dict(xbf=xbf, wn1t=wn1t, wn2t=wn2t, wentA=wentA, wentB=wentB,
                  bn1c=bn1c, bn2c=bn2c, we1m=we1m, wd1t=wd1t, wd2b=wd2b,
                  bd1p=bd1p, bd2p=bd2p, iota=iota, iotab=iotab,
                  ident=ident, identb=identb)
    in_maps = []
    for k in range(NC):
        m = dict(shared)
        m.update(xloc=xloc[k], up_src=up_src[k], up_dr=up_dr[k],
                 bidx=bidx[k], bsrel=bsrel[k], bstart=bstart[k],
                 bend=bend[k], degt=degt[k])
        in_maps.append(m)
    return in_maps, meta


def build_kernel(meta, stages=("up", "ag", "a", "b", "c")):
    stages = set(stages)
    cfg = meta["cfg"]
    P, N, M, E, C, FP, FN, NC = (cfg[k] for k in
                                 ("P", "N", "M", "E", "C", "FP", "FN", "NC"))
    M_LOC, NH, NHP, NMT = meta["M_LOC"], meta["NH"], meta["NHP"], meta["NMT"]
    NB, K_UP, NBK = meta["NB"], meta["K_UP"], meta["NBK"]
    KB, KBOFF = meta["kb"], meta["kboff"]
    G2 = meta["G2"]
    Kt, goff = meta["Kt"], meta["goff"]
    K_MAX = max(max(row) for row in Kt)
    CF = C * FP
    CN = C * FN
    assert C == 5

    nc = bacc.Bacc("TRN2", num_devices=NC, num_swdge_queues=4)

    def param(name, shape, dt=F32, out=False):
        return nc.declare_dram_parameter(name, list(shape), dt, isOutput=out)

    xbf_d = param("xbf", [P * N, CF], BF16)
    xloc_d = param("xloc", [P, 2, CF, NH], BF16)
    up_src_d = param("up_src", [P, 128, NBK], I32)
    up_dr_d = param("up_dr", [P, 128, NBK], BF16)
    bidx_d = param("bidx", [2 * P, 128, G2 * 8], I16)
    bsrel_d = param("bsrel", [2 * P, 128, G2], BF16)
    bstart_d = param("bstart", [2 * P, 128, G2], BF16)
    bend_d = param("bend", [2 * P, 128, G2], BF16)
    degt_d = param("degt", [2 * P, 128, NMT])
    wn1t_d = param("wn1t", [P, FP, CN])
    wn2t_d = param("wn2t", [FN, CN])
    wentA_d = param("wentA", [4 * FN, C * P])
    wentB_d = param("wentB", [FN + 1, C * P])
    bn1c_d = param("bn1c", [C, FN, 1])
    bn2c_d = param("bn2c", [C, FN, 1])
    we1m_d = param("we1m", [P, C, FP, C], BF16)
    wd1t_d = param("wd1t", [P, FP + FN, C * FP], BF16)
    wd2b_d = param("wd2b", [P, C, 2 * FP, FP], BF16)
    bd1p_d = param("bd1p", [P, 3, 2 * FP, 1])
    bd2p_d = param("bd2p", [P, 3, 2 * FP, 1])
    iota_d = param("iota", [128, 128])
    iotab_d = param("iotab", [128, 128], BF16)
    ident_d = param("ident", [128, 128])
    identb_d = param("identb", [128, 128], BF16)
    out_d = param("outT", [P, 2, C, FP, NHP], out=True)

    n_loc = nc.dram_tensor("n_loc", [M_LOC, NR], F32)
    n_full = nc.dram_tensor("n_full", [NC * M_LOC, NR], F32,
                            addr_space="Shared")
    # dma_gather cannot source from Shared address space and only moves
    # <=2-byte dtypes; mirror AG chunks into a plain DRAM tensor typed as
    # bf16 byte-pairs (same bytes) and gather from that.
    n_flat = nc.dram_tensor("n_flat", [NC * M_LOC, 2 * NR], BF16)

    PAIRS = ((0, 1), (2, 3), (4,))

    with tile.TileContext(nc) as tc:
        with tc.tile_pool(name="const", bufs=1) as cp:
            iota_t = cp.tile([128, 128], F32)
            nc.sync.dma_start(out=iota_t[:], in_=iota_d[:])
            iotab_t = cp.tile([128, 128], BF16)
            nc.sync.dma_start(out=iotab_t[:], in_=iotab_d[:])
            ident_t = cp.tile([128, 128], F32)
            nc.sync.dma_start(out=ident_t[:], in_=ident_d[:])
            identb_t = cp.tile([128, 128], BF16)
            nc.sync.dma_start(out=identb_t[:], in_=identb_d[:])
            wn1t_t = [cp.tile([FP, CN], F32R, name=f"wn1t{p}")
                      for p in range(P)]
            wn2t_t = cp.tile([FN, CN], F32R)
            wentA_t = cp.tile([4 * FN, C * P], F32R)
            wentB_t = cp.tile([FN + 1, C * P], F32R)
            nc.sync.dma_start(out=wn2t_t[:], in_=wn2t_d[:].bitcast(F32R))
            nc.sync.dma_start(out=wentA_t[:], in_=wentA_d[:].bitcast(F32R))
            nc.sync.dma_start(out=wentB_t[:], in_=wentB_d[:].bitcast(F32R))
            bn1c_t = [cp.tile([FN, 1], F32, name=f"bn1c{c}") for c in range(C)]
            bn2c_t = [cp.tile([FN, 1], F32, name=f"bn2c{c}") for c in range(C)]
            we1m_t = [[cp.tile([FP, C], BF16, name=f"we1m{p}_{c}")
                       for c in range(C)] for p in range(P)]
            wd1t_t = [cp.tile([FP + FN, C * FP], BF16, name=f"wd1t{p}")
                      for p in range(P)]
            wd2b_t = [[cp.tile([2 * FP, FP], BF16, name=f"wd2b{p}_{c}")
                       for c in range(C)] for p in range(P)]
            bd1p_t = [[cp.tile([2 * FP, 1], F32, name=f"bd1p{p}_{i}")
                       for i in range(3)] for p in range(P)]
            bd2p_t = [[cp.tile([2 * FP, 1], F32, name=f"bd2p{p}_{i}")
                       for i in range(3)] for p in range(P)]
            for p in range(P):
                nc.sync.dma_start(out=wn1t_t[p][:], in_=wn1t_d[p].bitcast(F32R))
                for c in range(C):
                    nc.sync.dma_start(out=we1m_t[p][c][:], in_=we1m_d[p, c])
                nc.sync.dma_start(out=wd1t_t[p][:], in_=wd1t_d[p])
                for c in range(C):
                    nc.sync.dma_start(out=wd2b_t[p][c][:], in_=wd2b_d[p, c])
                for i in range(3):
                    nc.sync.dma_start(out=bd1p_t[p][i][:], in_=bd1p_d[p, i])
                    nc.sync.dma_start(out=bd2p_t[p][i][:], in_=bd2p_d[p, i])
            for c in range(C):
                nc.sync.dma_start(out=bn1c_t[c][:], in_=bn1c_d[c])
                nc.sync.dma_start(out=bn2c_t[c][:], in_=bn2c_d[c])
            upsrc_t = [cp.tile([128, NBK], I32, name=f"upsrc{p}")
                       for p in range(P)]
            updr_t = [cp.tile([128, NBK], BF16, name=f"updr{p}")
                      for p in range(P)]
            for p in range(P):
                nc.scalar.dma_start(out=upsrc_t[p][:], in_=up_src_d[p])
                nc.scalar.dma_start(out=updr_t[p][:], in_=up_dr_d[p])
            bidx_t, bsrel_t, bstart_t, bend_t, degt_t = [], [], [], [], []
            for ph in range(2 * P):
                bi = cp.tile([128, G2 * 8], I16, name=f"bidx{ph}")
                nc.scalar.dma_start(out=bi[:], in_=bidx_d[ph])
                bidx_t.append(bi)
                s = cp.tile([128, G2], BF16, name=f"bsrel{ph}")
                nc.scalar.dma_start(out=s[:], in_=bsrel_d[ph])
                bsrel_t.append(s)
                s0 = cp.tile([128, G2], BF16, name=f"bstart{ph}")
                nc.scalar.dma_start(out=s0[:], in_=bstart_d[ph])
                bstart_t.append(s0)
                s1 = cp.tile([128, G2], BF16, name=f"bend{ph}")
                nc.scalar.dma_start(out=s1[:], in_=bend_d[ph])
                bend_t.append(s1)
                dg = cp.tile([128, NMT], F32, name=f"degt{ph}")
                nc.scalar.dma_start(out=dg[:], in_=degt_d[ph])
                degt_t.append(dg)
            ones_f = cp.tile([1, GRP * 128], F32)
            nc.vector.memset(ones_f[:], 1.0)
            ones_r = cp.tile([1, GRP * 128], F32R)
            nc.vector.tensor_copy(out=ones_r[:], in_=ones_f[:])

            # ======================= UP PHASE =======================
            n_loc_ap = n_loc.ap()
            with tc.tile_pool(name="up_sb", bufs=3) as up, \
                 tc.tile_pool(name="up_g", bufs=24) as upg, \
                 tc.tile_pool(name="up_sb1", bufs=2) as up1, \
                 tc.tile_pool(name="up_ps", bufs=2, space="PSUM") as upp, \
                 tc.tile_pool(name="up_ps1", bufs=1, space="PSUM") as upp1, \
                 tc.tile_pool(name="mlp_ps", bufs=1, space="PSUM") as mpp:
                uqr = [0]
                for g0 in (range(0, NB, GRP) if "up" in stages else []):
                    gb = list(range(g0, min(g0 + GRP, NB)))
                    GW = len(gb) * 128
                    upX = [[up1.tile([FP, GRP * 128], F32R,
                                     name=f"upX{p}_{c}", tag=f"upX{p}_{c}")
                            for c in range(C)] for p in range(P)]
                    for p in range(P):
                        for bi, b in enumerate(gb):
                            kbb = KB[p][b]
                            kb0 = KBOFF[p][b]
                            O8 = up.tile([128, K_UP, 128], BF16, tag="O8")
                            csl0 = slice(kb0, kb0 + kbb)
                            nc.vector.tensor_tensor(
                                out=O8[:, 0:kbb, :],
                                in0=updr_t[p][:, csl0]
                                    .rearrange("a (b c) -> a b c", c=1)
                                    .to_broadcast([128, kbb, 128]),
                                in1=iotab_t[:].rearrange("a (b c) -> a b c",
                                                         b=1)
                                    .to_broadcast([128, kbb, 128]),
                                op=ALU.is_equal)
                            pu = upp.tile([128, CF], F32, tag="pu",
                                          space="PSUM")
                            for kk in range(kbb):
                                col = kb0 + kk
                                G = upg.tile([128, CF], BF16, tag="G")
                                gi = nc.gpsimd.indirect_dma_start(
                                    out=G[:], out_offset=None,
                                    in_=xbf_d[:],
                                    in_offset=bass.IndirectOffsetOnAxis(
                                        ap=upsrc_t[p][:, col:col + 1], axis=0))
                                qi = uqr[0] % 4
                                uqr[0] += 1
                                if qi:
                                    gi.ins.queue = f"qPoolDynamic{qi}"
                                nc.tensor.matmul(out=pu[:],
                                                 lhsT=O8[:, kk, :],
                                                 rhs=G[:], start=(kk == 0),
                                                 stop=(kk == kbb - 1))
                            stg = up.tile([128, CF], F32, tag="stg")
                            nc.scalar.copy(out=stg[:], in_=pu[:])
                            csl = slice(bi * 128, (bi + 1) * 128)
                            for ti in range(3):
                                w = min(128, CF - ti * 128)
                                pt = upp1.tile([128, 128], F32, tag="ptr",
                                               space="PSUM")
                                nc.tensor.transpose(
                                    out=pt[:w, :],
                                    in_=stg[:, ti * 128:ti * 128 + w],
                                    identity=ident_t[:])
                                nc.vector.tensor_copy(
                                    out=upX[p][2 * ti][:, csl],
                                    in_=pt[0:FP, :])
                                if 2 * ti + 1 < C:
                                    nc.vector.tensor_copy(
                                        out=upX[p][2 * ti + 1][:, csl],
                                        in_=pt[FP:2 * FP, :])
                    # ---- nexus MLP over this group ----
                    n1c = [up.tile([FN, GRP * 128], F32R, name=f"n1c{c}",
                                   tag=f"n1c{c}") for c in range(C)]
                    for c in range(C):
                        pn1 = mpp.tile([FN, GRP * 128], F32, tag="pn1",
                                       space="PSUM", bufs=2)
                        for p in range(P):
                            nc.tensor.matmul(
                                out=pn1[:, :GW],
                                lhsT=wn1t_t[p][:, c * FN:(c + 1) * FN],
                                rhs=upX[p][c][:, :GW],
                                start=(p == 0), stop=(p == P - 1))
                        nc.scalar.activation(n1c[c][:, :GW], pn1[:, :GW],
                                             TANH, bias=bn1c_t[c][:])
                    n2s = up.tile([4 * FN, GRP * 128], F32R, tag="n2s")
                    nbt = up.tile([FN + 1, GRP * 128], F32R, tag="nbt")
                    nc.vector.tensor_copy(out=nbt[FN:FN + 1, :],
                                          in_=ones_r[:])
                    for c in range(C):
                        pn2 = mpp.tile([FN, GRP * 128], F32, tag="pn2",
                                       space="PSUM", bufs=2)
                        nc.tensor.matmul(
                            out=pn2[:, :GW],
                            lhsT=wn2t_t[:, c * FN:(c + 1) * FN],
                            rhs=n1c[c][:, :GW], start=True, stop=True)
                        dst = (n2s[c * FN:(c + 1) * FN, :GW] if c < 4
                               else nbt[0:FN, :GW])
                        nc.scalar.activation(dst, pn2[:, :GW],
                                             TANH, bias=bn2c_t[c][:])
                    pbv = mpp.tile([C * P, GRP * 128], F32, tag="misc",
                                   space="PSUM", bufs=1)
                    nc.tensor.matmul(out=pbv[:, :GW], lhsT=wentA_t[:],
                                     rhs=n2s[:, :GW], start=True, stop=False)
                    nc.tensor.matmul(out=pbv[:, :GW], lhsT=wentB_t[:],
                                     rhs=nbt[:, :GW], start=False, stop=True)
                    bt = up.tile([C * P, GRP * 128], F32, tag="bt")
                    nc.scalar.copy(out=bt[:, :GW], in_=pbv[:, :GW])
                    # assemble + store n rows per block
                    for bi, b in enumerate(gb):
                        rows = min(128, M_LOC - b * 128)
                        sl = slice(bi * 128, bi * 128 + 128)
                        tp = mpp.tile([128, CN + C * P], F32,
                                      tag="misc", space="PSUM", bufs=1)
                        nc.tensor.transpose(
                            out=tp[:, 0:4 * FN],
                            in_=n2s[:, sl].bitcast(F32),
                            identity=ident_t[:])
                        nc.tensor.transpose(
                            out=tp[:, 4 * FN:CN],
                            in_=nbt[0:FN, sl].bitcast(F32),
                            identity=ident_t[:FN, :FN])
                        nc.tensor.transpose(
                            out=tp[:, CN:CN + C * P],
                            in_=bt[:, sl],
                            identity=ident_t[:C * P, :C * P])
                        nrow = up.tile([128, NR], F32, tag="nrow")
                        nc.scalar.copy(out=nrow[:, 0:CN + C * P], in_=tp[:])
                        nc.vector.memset(nrow[:, CN + C * P:], 0.0)
                        nc.sync.dma_start(
                            out=n_loc_ap[b * 128:b * 128 + rows, :],
                            in_=nrow[:rows, :])
                    if "ag" in stages:
                        CH = GRP * 128
                        jch = g0 // GRP
                        lo = jch * CH
                        hi = min(lo + CH, M_LOC)
                        if hi > lo:
                            base = NC * CH * jch
                            nrows = NC * (hi - lo)
                            nfv = n_full.ap()[base:base + nrows, :]
                            nc.gpsimd.collective_compute(
                                "AllGather", ALU.bypass,
                                replica_groups=[list(range(NC))],
                                ins=[n_loc_ap[lo:hi, :].opt()],
                                outs=[nfv.opt()])
                            nc.sync.dma_start(
                                out=n_flat.ap()[base:base + nrows, :],
                                in_=n_full.ap()[base:base + nrows, :]
                                .bitcast(BF16))

            # ================= AllGather n (ablation fallback) ==========
            if "ag" in stages and "up" not in stages:
                CH = GRP * 128
                J_AG = _ceil(M_LOC, CH)
                for jch in range(J_AG):
                    lo = jch * CH
                    hi = min(lo + CH, M_LOC)
                    base = NC * CH * jch
                    nrows = NC * (hi - lo)
                    nc.gpsimd.collective_compute(
                        "AllGather", ALU.bypass,
                        replica_groups=[list(range(NC))],
                        ins=[n_loc.ap()[lo:hi, :].opt()],
                        outs=[n_full.ap()[base:base + nrows, :].opt()])
                    nc.sync.dma_start(
                        out=n_flat.ap()[base:base + nrows, :],
                        in_=n_full.ap()[base:base + nrows, :].bitcast(BF16))

            # ============ FUSED A+B+C per (plane, half) ============
            NG = max(10, K_MAX)
            TBMAX = 4
            qrr = [0]
            packs = {}                    # ph -> list of (t0, tbw)
            for ph0 in range(2 * P):
                lst = []
                t = 0
                while t < NMT:
                    tw, gsum = 0, 0
                    while (t + tw < NMT and tw < TBMAX
                           and (tw == 0
                                or gsum + Kt[ph0][t + tw] <= NG)):
                        gsum += Kt[ph0][t + tw]
                        tw += 1
                    lst.append((t, tw))
                    t += tw
                packs[ph0] = lst
            with tc.tile_pool(name="ft_sb", bufs=1) as ftp, \
                 tc.tile_pool(name="ab_sb", bufs=2) as ab, \
                 tc.tile_pool(name="gn_sb", bufs=5) as gnp, \
                 tc.tile_pool(name="b_sb", bufs=2) as sbp, \
                 tc.tile_pool(name="b_ps", bufs=1, space="PSUM") as bps, \
                 tc.tile_pool(name="agg_ps", bufs=1, space="PSUM") as agp, \
                 tc.tile_pool(name="c_sb", bufs=2) as scb, \
                 tc.tile_pool(name="c_ps", bufs=2, space="PSUM") as scp:
                for ph in range(2 * P):
                    p, h = ph // 2, ph % 2
                    # ---- stage A: load x into ft (bf16), aT, a_sb ----
                    ft = [ftp.tile([FP + FN, NHP], BF16, name=f"ft{c}",
                                   tag=f"ft{c}") for c in range(C)]
                    a_sb = ab.tile([128, NMT, 8], BF16, tag="a_sb")
                    if "a" in stages:
                        for c in range(C):
                            if NHP > NH:
                                nc.vector.memset(ft[c][0:FP, NH:], 0.0)
                            nc.sync.dma_start(
                                out=ft[c][0:FP, :NH],
                                in_=xloc_d[p, h, c * FP:(c + 1) * FP, :])
                        with nc.allow_low_precision(reason="invdeg bf16"):
                            nc.vector.reciprocal(
                                out=a_sb[:, :, 5:6],
                                in_=degt_t[ph][:]
                                .rearrange("a (b c) -> a b c", c=1))
                        for t0 in range(0, NMT, 8):
                            tw = min(8, NMT - t0)
                            pa = bps.tile([128, NG, 6], F32, tag="pa8",
                                          space="PSUM")
                            for ti in range(tw):
                                t = t0 + ti
                                tsl = slice(t * 128, (t + 1) * 128)
                                for c in range(C):
                                    nc.tensor.matmul(
                                        out=pa[:, ti, 0:C],
                                        lhsT=ft[c][0:FP, tsl],
                                        rhs=we1m_t[p][c][:],
                                        start=(c == 0), stop=(c == C - 1))
                            nc.vector.tensor_copy(
                                out=a_sb[:, t0:t0 + tw, 0:5],
                                in_=pa[:, 0:tw, 0:C])
                    # ---- stage B ----
                    if "b" in stages:
                        for (t0, tbw) in packs[ph]:
                            g0 = goff[ph][t0]
                            gw = goff[ph][t0 + tbw] - g0
                            gsl = slice(g0, g0 + gw)
                            if gw == 0:
                                tsl0 = slice(t0 * 128, (t0 + tbw) * 128)
                                for c in range(C):
                                    nc.vector.memset(
                                        ft[c][FP:FP + FN, tsl0], 0.0)
                                continue
                            gn = gnp.tile([128, NG, NR], F32, tag="gn")
                            gnb = gn[:].bitcast(BF16)
                            for go in range(0, gw, GMAX):
                                gww = min(GMAX, gw - go)
                                nc.gpsimd.dma_gather(
                                    out_ap=gnb[:, go:go + gww, :],
                                    in_ap=n_flat.ap()[:],
                                    idxs_ap=bidx_t[ph][
                                        :, (g0 + go) * 8:(g0 + go + gww) * 8],
                                    num_idxs=gww * 128,
                                    num_idxs_reg=gww * 128,
                                    elem_size=2 * NR,
                                    single_packet=True,
                                    queue_num=qrr[0] % 4)
                                qrr[0] += 1
                            Oag = sbp.tile([128, NG, 128], BF16, tag="Oag")
                            nc.vector.tensor_tensor(
                                out=Oag[:, :gw, :],
                                in0=bsrel_t[ph][:, gsl]
                                    .rearrange("a (b c) -> a b c", c=1)
                                    .to_broadcast([128, gw, 128]),
                                in1=iotab_t[:].rearrange("a (b c) -> a b c",
                                                         b=1)
                                    .to_broadcast([128, gw, 128]),
                                op=ALU.is_equal)
                            Oge = sbp.tile([128, NG, 128], BF16, tag="Oge")
                            nc.vector.tensor_tensor(
                                out=Oge[:, :gw, :],
                                in0=iotab_t[:].rearrange("a (b c) -> a b c",
                                                         b=1)
                                    .to_broadcast([128, gw, 128]),
                                in1=bstart_t[ph][:, gsl]
                                    .rearrange("a (b c) -> a b c", c=1)
                                    .to_broadcast([128, gw, 128]),
                                op=ALU.is_ge)
                            Obc = sbp.tile([128, NG, 128], BF16, tag="Obc")
                            nc.vector.tensor_tensor(
                                out=Obc[:, :gw, :],
                                in0=iotab_t[:].rearrange("a (b c) -> a b c",
                                                         b=1)
                                    .to_broadcast([128, gw, 128]),
                                in1=bend_t[ph][:, gsl]
                                    .rearrange("a (b c) -> a b c", c=1)
                                    .to_broadcast([128, gw, 128]),
                                op=ALU.is_lt)
                            nc.vector.tensor_tensor(
                                out=Obc[:, :gw, :], in0=Obc[:, :gw, :],
                                in1=Oge[:, :gw, :], op=ALU.mult)
                            pa8 = bps.tile([128, NG, 6], F32, tag="pa8",
                                           space="PSUM")
                            for ti in range(tbw):
                                t = t0 + ti
                                for jj2 in range(Kt[ph][t]):
                                    jj = goff[ph][t] - g0 + jj2
                                    nc.tensor.matmul(
                                        out=pa8[:, jj, :],
                                        lhsT=Obc[:, jj, :],
                                        rhs=a_sb[:, t, 0:6],
                                        start=True, stop=True)
                            lg = sbp.tile([128, NG, C], F32, tag="lg")
                            nc.vector.tensor_tensor(
                                out=lg[:, :gw, :], in0=pa8[:, :gw, 0:5],
                                in1=gn[:, :gw, CN + p:CN + p
                                       + (C - 1) * P + 1:P],
                                op=ALU.add)
                            ex = sbp.tile([128, NG, C], F32, tag="ex")
                            nc.scalar.activation(ex[:, :gw, :], lg[:, :gw, :],
                                                 EXP)
                            sm = sbp.tile([128, NG], F32, tag="sm")
                            nc.vector.tensor_reduce(
                                out=sm[:, :gw], in_=ex[:, :gw, :],
                                axis=mybir.AxisListType.X, op=ALU.add)
                            nc.vector.reciprocal(out=sm[:, :gw],
                                                 in_=sm[:, :gw])
                            nc.vector.tensor_tensor(
                                out=sm[:, :gw], in0=sm[:, :gw],
                                in1=pa8[:, :gw, 5], op=ALU.mult)
                            nc.vector.tensor_tensor(
                                out=ex[:, :gw, :], in0=ex[:, :gw, :],
                                in1=sm[:, :gw].rearrange("a (b c) -> a b c",
                                                         c=1)
                                    .to_broadcast([128, gw, C]),
                                op=ALU.mult)
                            msg = sbp.tile([128, NG, CN], BF16, tag="msg")
                            nc.vector.tensor_tensor(
                                out=msg[:, :gw, :].rearrange(
                                    "a b (c f) -> a b c f", f=FN),
                                in0=gn[:, :gw, 0:CN].rearrange(
                                    "a b (c f) -> a b c f", f=FN),
                                in1=ex[:, :gw, :].rearrange(
                                    "a b (c d) -> a b c d", d=1)
                                    .to_broadcast([128, gw, C, FN]),
                                op=ALU.mult)
                            psA = agp.tile([128, TBMAX * 128], F32,
                                           tag="psA", space="PSUM")
                            psB = agp.tile([FN, TBMAX * 128], F32,
                                           tag="psB", space="PSUM")
                            nzw = 0
                            for ti in range(tbw):
                                t = t0 + ti
                                kt = Kt[ph][t]
                                if kt == 0:
                                    break
                                nzw += 1
                                bsl = slice(ti * 128, (ti + 1) * 128)
                                for jj2 in range(kt):
                                    jj = goff[ph][t] - g0 + jj2
                                    nc.tensor.matmul(
                                        out=psA[:, bsl],
                                        lhsT=msg[:, jj, 0:128],
                                        rhs=Oag[:, jj, :],
                                        start=(jj2 == 0),
                                        stop=(jj2 == kt - 1))
                                for jj2 in range(kt):
                                    jj = goff[ph][t] - g0 + jj2
                                    nc.tensor.matmul(
                                        out=psB[:, bsl],
                                        lhsT=msg[:, jj, 128:CN],
                                        rhs=Oag[:, jj, :],
                                        start=(jj2 == 0),
                                        stop=(jj2 == kt - 1))
                            csl2 = slice(t0 * 128, (t0 + nzw) * 128)
                            if nzw > 0:
                                for c in range(4):
                                    nc.scalar.copy(
                                        out=ft[c][FP:FP + FN, csl2],
                                        in_=psA[c * FN:(c + 1) * FN,
                                                0:nzw * 128])
                                nc.scalar.copy(
                                    out=ft[4][FP:FP + FN, csl2],
                                    in_=psB[:, 0:nzw * 128])
                            if nzw < tbw:
                                zsl = slice((t0 + nzw) * 128,
                                            (t0 + tbw) * 128)
                                for c in range(C):
                                    nc.vector.memset(
                                        ft[c][FP:FP + FN, zsl], 0.0)
                    # ---- stage C: down MLP (bf16, class pairs) ----
                    if "c" in stages:
                        for ch0 in range(0, NHP, CHW):
                            cw = min(CHW, NHP - ch0)
                            csl = slice(ch0, ch0 + cw)
                            for pi, pr in enumerate(PAIRS):
                                rows = len(pr) * FP
                                hps = scp.tile([2 * FP, CHW], F32, tag="hps",
                                               space="PSUM")
                                for i, c in enumerate(pr):
                                    nc.tensor.matmul(
                                        out=hps[i * FP:(i + 1) * FP, :cw],
                                        lhsT=wd1t_t[p][:, c * FP:(c + 1) * FP],
                                        rhs=ft[c][:, csl],
                                        start=True, stop=True)
                                ht = scb.tile([2 * FP, CHW], BF16, tag="ht")
                                nc.scalar.activation(
                                    ht[0:rows, :cw], hps[0:rows, :cw],
                                    TANH, bias=bd1p_t[p][pi][0:rows])
                                ops_ = scp.tile([2 * FP, CHW], F32, tag="ops",
                                                space="PSUM")
                                for i, c in enumerate(pr):
                                    nc.tensor.matmul(
                                        out=ops_[i * FP:(i + 1) * FP, :cw],
                                        lhsT=wd2b_t[p][c][
                                            i * FP:(i + 1) * FP, :],
                                        rhs=ht[i * FP:(i + 1) * FP, :cw],
                                        start=True, stop=True)
                                ot = scb.tile([2 * FP, CHW], F32, tag="ot")
                                nc.scalar.activation(
                                    ot[0:rows, :cw], ops_[0:rows, :cw],
                                    TANH, bias=bd2p_t[p][pi][0:rows])
                                for i, c in enumerate(pr):
                                    nc.sync.dma_start(
                                        out=out_d[p, h, c, :, csl],
                                        in_=ot[i * FP:(i + 1) * FP, :cw])

    nc.compile()
    return nc


_CACHE = {}


def _get_compiled(inputs, cfg):
    in_maps, meta = host_prep(inputs, cfg)
    key = (meta["K_UP"], meta["Kt"], tuple(sorted(cfg.items())))
    if key not in _CACHE:
        _CACHE[key] = build_kernel(meta)
    return _CACHE[key], in_maps, meta


def assemble_output(results, meta):
    cfg = meta["cfg"]
    P, N, C, FP, NC = (cfg[k] for k in ("P", "N", "C", "FP", "NC"))
    NH = meta["NH"]
    pc, phh, pl = meta["perm_core"], meta["perm_h"], meta["perm_loc"]
    arr = np.stack([np.asarray(results[k]["outT"])[:, :, :, :, :NH]
                    for k in range(NC)])      # [NC, P, 2, C, FP, NH]
    out = np.empty((P, N, C, FP), np.float32)
    for p in range(P):
        out[p] = arr[pc[p], p, phh[p], :, :, pl[p]]
    return np.ascontiguousarray(out)


def kernel(**inputs):
    from concourse.bass_utils import run_bass_kernel_spmd
    cfg = CFG_FULL
    nc, in_maps, meta = _get_compiled(inputs, cfg)
    res = run_bass_kernel_spmd(nc, in_maps, list(range(cfg["NC"])))
    return assemble_output(res.results, meta)


# revision 37
# speedup vs baseline: 1.6920x; 1.0495x over previous
"""NexusNet GNN message-passing kernel v3 for 8 Trainium2 NeuronCores.

Sharding:
  - nexus_up + nexus MLP: sharded by nexus node (M/8 contiguous segs/core);
    edges routed to the core owning their dst segment (host index prep).
    x gathered from a bf16 table via 128-row indirect DMAs; aggregation via
    one-hot matmul on PE into PSUM per 128-seg block.
  - n [M,192] f32 rows (160 n + 15 b + pad) AllGathered to every core.
  - down: sharded by planar node (N/8 per core, 2 halves/core/plane), fused
    A+B+C per (plane,half), bf16 compute:
      A: x loaded bf16 feature-major into ft; per-node per-class logits via
         PSUM-accumulated matmuls against class-masked we1 (node-major a_sb
         directly); invdeg folded in.
      B: per pack of src blocks: batched dma_gather (int16 idx, round-robin
         SWDGE queues) of the pack's edge n-rows from a non-shared mirror of
         the AllGathered table; softmax weights (no max-subtract; logits are
         bounded); one-hot matmul aggregates messages feature-major into ft.
      C: 2-layer down MLP bf16, classes processed in pairs packed into the
         full 128 PSUM partitions; output transposed on host.

  SWDGE notes (hard-won): descriptor generation costs ~8.5ns/row on the Q7
  regardless of batching; dma_gather crashes on >1024 idx with
  single_packet=True (64-desc packet cap), on fp32 sources, and is limited
  to int16 row indices; collectives must write Shared space but dma_gather
  must read non-Shared, hence the n_flat mirror.
"""

import numpy as np

import concourse.bass as bass
import concourse.bacc as bacc
import concourse.mybir as mybir
import concourse.tile as tile

F32 = mybir.dt.float32
F32R = mybir.dt.float32r
BF16 = mybir.dt.bfloat16
I32 = mybir.dt.int32
I16 = mybir.dt.int16
TANH = mybir.ActivationFunctionType.Tanh
EXP = mybir.ActivationFunctionType.Exp
ALU = mybir.AluOpType

CFG_FULL = dict(P=3, N=100000, M=30000, E=200000, C=5, FP=64, FN=32, NC=8)

GRP = 4               # up-phase seg blocks per nexus-MLP group
CHW = 512             # down-MLP chunk width (4 src blocks)
NR = 192              # n-row floats (160 n + 15 b + 17 pad); 768B %256==0
GMAX = 8              # max 128-idx columns per dma_gather call (ring limit)


def _ceil(a, b):
    return (a + b - 1) // b


def _wrap_idx16(idx, ncols):
    """Flat row-index list -> [128, ncols] int16 wrapped (i%16, i//16),
    replicated across the 8 gpsimd cores."""
    n = len(idx)
    a = np.zeros((16, ncols), np.int16)
    a[np.arange(n) % 16, np.arange(n) // 16] = idx.astype(np.int16)
    return np.tile(a, (8, 1))


def host_prep(inputs, cfg):
    P, N, M, E, C, FP, FN, NC = (cfg[k] for k in
                                 ("P", "N", "M", "E", "C", "FP", "FN", "NC"))
    CF = C * FP
    CN = C * FN
    M_LOC = M // NC
    N_LOC = N // NC
    NH = N_LOC // 2                       # nodes per half
    NHP = _ceil(NH, 128) * 128            # padded half
    NMT = NHP // 128                      # src blocks per half
    NB = _ceil(M_LOC, 128)                # up seg blocks per core

    x = np.ascontiguousarray(np.asarray(inputs["x"], np.float32)
                             .reshape(P, N, CF))
    esrc = np.asarray(inputs["edge_src"])
    edst = np.asarray(inputs["edge_dst"])

    bfnp = mybir.dt.np(BF16)
    xbf = np.ascontiguousarray(x.reshape(P * N, CF)).astype(bfnp)

    # ---------------- UP phase indices ----------------
    per_kp = {}
    max_blk_cnt = 0
    for p in range(P):
        order = np.argsort(edst[p], kind="stable")
        ds, ss = edst[p][order], esrc[p][order]
        bounds = np.searchsorted(ds, np.arange(NC + 1) * M_LOC)
        for k in range(NC):
            sl = slice(bounds[k], bounds[k + 1])
            dsl = (ds[sl] - k * M_LOC).astype(np.int64)
            blk = dsl >> 7
            cnt = np.bincount(blk, minlength=NB)
            max_blk_cnt = max(max_blk_cnt, int(cnt.max(initial=0)))
            per_kp[(k, p)] = (dsl, (ss[sl] + p * N).astype(np.int64), blk, cnt)
    K_UP = max(1, _ceil(max_blk_cnt, 128))
    # per-(p, b) column count: max over cores (SPMD program is shared)
    kb = np.ones((P, NB), np.int64)
    for (k, p), (dsl, sglob, blk, cnt) in per_kp.items():
        kb[p] = np.maximum(kb[p], (cnt + 127) // 128)
    kboff = np.zeros((P, NB + 1), np.int64)
    kboff[:, 1:] = np.cumsum(kb, axis=1)
    NBK = int(kboff[:, -1].max())

    up_src = np.zeros((NC, P, NBK * 128), np.int32)
    up_dr = np.full((NC, P, NBK * 128), -1.0, np.float32)
    for (k, p), (dsl, sglob, blk, cnt) in per_kp.items():
        starts = np.concatenate(([0], np.cumsum(cnt)))[:-1]
        r = np.arange(len(dsl)) - np.repeat(starts, cnt)
        pos = kboff[p][blk] * 128 + r
        up_src[k, p, pos] = sglob
        up_dr[k, p, pos] = dsl - (blk << 7)
    up_src = up_src.reshape(NC, P, NBK, 128).transpose(0, 1, 3, 2).copy()
    up_dr = (up_dr.reshape(NC, P, NBK, 128).transpose(0, 1, 3, 2)
             .astype(bfnp).copy())

    # ---------------- DOWN (stage B) indices ----------------
    # Per-plane striped degree-sort relabel: sorted rank r -> core r%NC,
    # local slot r//NC (-> half, loc).
    perm_core = np.empty((P, N), np.int64)
    perm_h = np.empty((P, N), np.int64)
    perm_loc = np.empty((P, N), np.int64)
    edges = {}
    for p in range(P):
        degp = np.bincount(esrc[p], minlength=N)
        orderp = np.argsort(-degp, kind="stable")
        rank = np.empty(N, np.int64)
        rank[orderp] = np.arange(N)
        loc = rank // NC
        perm_core[p] = rank % NC
        perm_h[p] = loc // NH
        perm_loc[p] = loc % NH
        ec, eh = perm_core[p][esrc[p]], perm_h[p][esrc[p]]
        el, ed = perm_loc[p][esrc[p]], edst[p].astype(np.int64)
        for k in range(NC):
            for h in range(2):
                sel = (ec == k) & (eh == h)
                s_loc = el[sel]
                d_loc = ed[sel]
                o2 = np.argsort(s_loc, kind="stable")
                edges[(k, p, h)] = (s_loc[o2], d_loc[o2])
    # chunk-major n_full layout: AllGather chunk j (CH local rows) lands
    # contiguously at NC*CH*j; nexus id m -> (core m//M_LOC, loc m%M_LOC)
    CH = GRP * 128
    J_AG = _ceil(M_LOC, CH)
    lastch = M_LOC - (J_AG - 1) * CH

    def g2r(m):
        k_ = m // M_LOC
        r_ = m % M_LOC
        j_ = np.minimum(r_ // CH, J_AG - 1)
        base = NC * CH * j_
        chs = np.where(j_ < J_AG - 1, CH, lastch)
        return base + k_ * chs + (r_ - CH * j_)

    # profiles
    Kt = np.zeros((2 * P, NMT), np.int64)
    cnts = {}
    for (k, p, h), (s_loc, d_loc) in edges.items():
        ph = p * 2 + h
        cnt = np.bincount(s_loc >> 7, minlength=NMT)
        cnts[(k, p, h)] = cnt
        Kt[ph] = np.maximum(Kt[ph], (cnt + 127) // 128)
    goff = np.zeros((2 * P, NMT + 1), np.int64)
    goff[:, 1:] = np.cumsum(Kt, axis=1)
    G2 = int(goff[:, -1].max())

    bdst = np.zeros((NC, 2 * P, 128, G2), np.int64)
    bsrel = np.full((NC, 2 * P, 128, G2), -1.0, bfnp)
    bstart = np.zeros((NC, 2 * P, 128, G2), bfnp)
    bend = np.zeros((NC, 2 * P, 128, G2), bfnp)
    degt = np.ones((NC, 2 * P, 128, NMT), np.float32)
    for (k, p, h), (s_loc, d_loc) in edges.items():
        ph = p * 2 + h
        cnt = cnts[(k, p, h)]
        deg = np.bincount(s_loc, minlength=NHP)
        degt[k, ph, :, :] = np.maximum(
            deg.reshape(NMT, 128).T, 1.0).astype(np.float32)
        bb = np.concatenate(([0], np.cumsum(cnt)))
        for t in range(NMT):
            seg_s = s_loc[bb[t]:bb[t + 1]] - 128 * t
            seg_d = d_loc[bb[t]:bb[t + 1]]
            n_t = len(seg_s)
            cum = np.concatenate(
                ([0], np.cumsum(np.bincount(seg_s, minlength=128))))
            for jj in range(int(Kt[ph][t])):
                g = int(goff[ph][t]) + jj
                lo = jj * 128
                nh_ = min(max(n_t - lo, 0), 128)
                if nh_ > 0:
                    bdst[k, ph, :nh_, g] = g2r(seg_d[lo:lo + nh_])
                    bsrel[k, ph, :nh_, g] = seg_s[lo:lo + nh_]
                bstart[k, ph, :, g] = np.clip(cum[:128] - lo, 0, 128)
                bend[k, ph, :, g] = np.clip(cum[1:] - lo, 0, 128)
    # int16 wrapped gather indices: slot (jj*128 + j) at (s%16, s//16), x8
    bidx = np.zeros((NC, 2 * P, 128, G2 * 8), np.int16)
    for k in range(NC):
        for ph in range(2 * P):
            flat = bdst[k, ph].T.reshape(-1)          # [G2*128] slot-major
            bidx[k, ph] = _wrap_idx16(flat, G2 * 8)

    # per-core feature-major bf16 x slices after relabel: [NC, P, 2, CF, NH]
    inv = np.empty((P, NC, 2, NH), np.int64)
    for p in range(P):
        inv[p, perm_core[p], perm_h[p], perm_loc[p]] = np.arange(N)
    xloc = np.empty((NC, P, 2, CF, NH), bfnp)
    for p in range(P):
        for k in range(NC):
            for h in range(2):
                xloc[k, p, h] = x[p][inv[p, k, h]].T.astype(bfnp)

    # ---------------- weights ----------------
    g = lambda n: np.asarray(inputs[n], np.float32)
    Wn1, Wn2, We, Wd1, Wd2 = g("Wn1"), g("Wn2"), g("We"), g("Wd1"), g("Wd2")
    bn1, bn2, be, bd1, bd2 = g("bn1"), g("bn2"), g("be"), g("bd1"), g("bd2")

    wn1t = np.stack([Wn1.transpose(2, 0, 1)[p * FP:(p + 1) * FP]
                     .reshape(FP, C * FN) for p in range(P)]).copy()
    wn2t = Wn2.transpose(2, 0, 1).reshape(FN, C * FN).copy()
    went = We[:, :, 0, FP:]                                   # [P, C, FN]
    wentA = np.zeros((4 * FN, C * P), np.float32)
    for c in range(4):
        wentA[c * FN:(c + 1) * FN, c * P:(c + 1) * P] = went[:, c, :].T
    wentB = np.zeros((FN + 1, C * P), np.float32)
    wentB[:FN, 4 * P:] = went[:, 4, :].T
    wentB[FN, :] = be[:, :, 0].T.reshape(-1)
    bn1c = bn1.reshape(C, FN, 1).copy()
    bn2c = bn2.reshape(C, FN, 1).copy()
    # per-class masked we1: we1m[p, c, :, c'] = We[p,c,0,:FP] iff c'==c.
    # Accumulating the 5 per-class matmuls in PSUM yields a[node, 0:5].
    we1m = np.zeros((P, C, FP, C), np.float32)
    for c in range(C):
        we1m[:, c, :, c] = We[:, c, 0, :FP]
    we1m = we1m.astype(bfnp)
    wd1t = (Wd1.transpose(0, 3, 1, 2).reshape(P, FP + FN, C * FP)
            .astype(bfnp).copy())
    # wd2 duplicated at partition bases 0 and 64 (PE needs lhsT/rhs bases
    # to match; the paired stage-C rhs lives at base 0 or 64).
    wd2t = Wd2.transpose(0, 1, 3, 2).astype(bfnp)             # [P, C, FP, FP]
    wd2b = np.concatenate([wd2t, wd2t], axis=2).copy()        # [P,C,2FP,FP]
    # class-pair packed biases: pairs (0,1), (2,3), (4,)
    bd1p = np.zeros((P, 3, 2 * FP, 1), np.float32)
    bd2p = np.zeros((P, 3, 2 * FP, 1), np.float32)
    for pi, pr in enumerate(((0, 1), (2, 3), (4,))):
        for i, c in enumerate(pr):
            bd1p[:, pi, i * FP:(i + 1) * FP, 0] = bd1[:, c, :]
            bd2p[:, pi, i * FP:(i + 1) * FP, 0] = bd2[:, c, :]
    iota = np.tile(np.arange(128, dtype=np.float32), (128, 1)).copy()
    iotab = iota.astype(bfnp)
    ident = np.eye(128, dtype=np.float32)
    identb = np.eye(128, dtype=np.float32).astype(bfnp)

    meta = dict(cfg=cfg, M_LOC=M_LOC, N_LOC=N_LOC, NH=NH, NHP=NHP,
                NMT=NMT, NB=NB, K_UP=K_UP, NBK=NBK, G2=G2,
                kb=tuple(tuple(int(v) for v in row) for row in kb),
                kboff=tuple(tuple(int(v) for v in row) for row in kboff),
                Kt=tuple(tuple(int(v) for v in row) for row in Kt),
                goff=tuple(tuple(int(v) for v in row) for row in goff),
                perm_core=perm_core, perm_h=perm_h, perm_loc=perm_loc)

    shared = dict(xbf=xbf, wn1t=wn1t, wn2t=wn2t, wentA=wentA, wentB=wentB,
                  bn1c=bn1c, bn2c=bn2c, we1m=we1m, wd1t=wd1t, wd2b=wd2b,
                  bd1p=bd1p, bd2p=bd2p, iota=iota, iotab=iotab,
                  ident=ident, identb=identb)
    in_maps = []
    for k in range(NC):
        m = dict(shared)
        m.update(xloc=xloc[k], up_src=up_src[k], up_dr=up_dr[k],
                 bidx=bidx[k], bsrel=bsrel[k], bstart=bstart[k],
                 bend=bend[k], degt=degt[k])
        in_maps.append(m)
    return in_maps, meta


def build_kernel(meta, stages=("up", "ag", "a", "b", "c")):
    stages = set(stages)
    cfg = meta["cfg"]
    P, N, M, E, C, FP, FN, NC = (cfg[k] for k in
                                 ("P", "N", "M", "E", "C", "FP", "FN", "NC"))
    M_LOC, NH, NHP, NMT = meta["M_LOC"], meta["NH"], meta["NHP"], meta["NMT"]
    NB, K_UP, NBK = meta["NB"], meta["K_UP"], meta["NBK"]
    KB, KBOFF = meta["kb"], meta["kboff"]
    G2 = meta["G2"]
    Kt, goff = meta["Kt"], meta["goff"]
    K_MAX = max(max(row) for row in Kt)
    CF = C * FP
    CN = C * FN
    assert C == 5

    nc = bacc.Bacc("TRN2", num_devices=NC, num_swdge_queues=4)

    def param(name, shape, dt=F32, out=False):
        return nc.declare_dram_parameter(name, list(shape), dt, isOutput=out)

    xbf_d = param("xbf", [P * N, CF], BF16)
    xloc_d = param("xloc", [P, 2, CF, NH], BF16)
    up_src_d = param("up_src", [P, 128, NBK], I32)
    up_dr_d = param("up_dr", [P, 128, NBK], BF16)
    bidx_d = param("bidx", [2 * P, 128, G2 * 8], I16)
    bsrel_d = param("bsrel", [2 * P, 128, G2], BF16)
    bstart_d = param("bstart", [2 * P, 128, G2], BF16)
    bend_d = param("bend", [2 * P, 128, G2], BF16)
    degt_d = param("degt", [2 * P, 128, NMT])
    wn1t_d = param("wn1t", [P, FP, CN])
    wn2t_d = param("wn2t", [FN, CN])
    wentA_d = param("wentA", [4 * FN, C * P])
    wentB_d = param("wentB", [FN + 1, C * P])
    bn1c_d = param("bn1c", [C, FN, 1])
    bn2c_d = param("bn2c", [C, FN, 1])
    we1m_d = param("we1m", [P, C, FP, C], BF16)
    wd1t_d = param("wd1t", [P, FP + FN, C * FP], BF16)
    wd2b_d = param("wd2b", [P, C, 2 * FP, FP], BF16)
    bd1p_d = param("bd1p", [P, 3, 2 * FP, 1])
    bd2p_d = param("bd2p", [P, 3, 2 * FP, 1])
    iota_d = param("iota", [128, 128])
    iotab_d = param("iotab", [128, 128], BF16)
    ident_d = param("ident", [128, 128])
    identb_d = param("identb", [128, 128], BF16)
    out_d = param("outT", [P, 2, C, FP, NHP], out=True)

    n_loc = nc.dram_tensor("n_loc", [M_LOC, NR], F32)
    n_full = nc.dram_tensor("n_full", [NC * M_LOC, NR], F32,
                            addr_space="Shared")
    # dma_gather cannot source from Shared address space and only moves
    # <=2-byte dtypes; mirror AG chunks into a plain DRAM tensor typed as
    # bf16 byte-pairs (same bytes) and gather from that.
    n_flat = nc.dram_tensor("n_flat", [NC * M_LOC, 2 * NR], BF16)

    PAIRS = ((0, 1), (2, 3), (4,))

    with tile.TileContext(nc) as tc:
        with tc.tile_pool(name="const", bufs=1) as cp:
            iota_t = cp.tile([128, 128], F32)
            nc.sync.dma_start(out=iota_t[:], in_=iota_d[:])
            iotab_t = cp.tile([128, 128], BF16)
            nc.sync.dma_start(out=iotab_t[:], in_=iotab_d[:])
            ident_t = cp.tile([128, 128], F32)
            nc.sync.dma_start(out=ident_t[:], in_=ident_d[:])
            identb_t = cp.tile([128, 128], BF16)
            nc.sync.dma_start(out=identb_t[:], in_=identb_d[:])
            wn1t_t = [cp.tile([FP, CN], F32R, name=f"wn1t{p}")
                      for p in range(P)]
            wn2t_t = cp.tile([FN, CN], F32R)
            wentA_t = cp.tile([4 * FN, C * P], F32R)
            wentB_t = cp.tile([FN + 1, C * P], F32R)
            nc.sync.dma_start(out=wn2t_t[:], in_=wn2t_d[:].bitcast(F32R))
            nc.sync.dma_start(out=wentA_t[:], in_=wentA_d[:].bitcast(F32R))
            nc.sync.dma_start(out=wentB_t[:], in_=wentB_d[:].bitcast(F32R))
            bn1c_t = [cp.tile([FN, 1], F32, name=f"bn1c{c}") for c in range(C)]
            bn2c_t = [cp.tile([FN, 1], F32, name=f"bn2c{c}") for c in range(C)]
            we1m_t = [[cp.tile([FP, C], BF16, name=f"we1m{p}_{c}")
                       for c in range(C)] for p in range(P)]
            wd1t_t = [cp.tile([FP + FN, C * FP], BF16, name=f"wd1t{p}")
                      for p in range(P)]
            wd2b_t = [[cp.tile([2 * FP, FP], BF16, name=f"wd2b{p}_{c}")
                       for c in range(C)] for p in range(P)]
            bd1p_t = [[cp.tile([2 * FP, 1], F32, name=f"bd1p{p}_{i}")
                       for i in range(3)] for p in range(P)]
            bd2p_t = [[cp.tile([2 * FP, 1], F32, name=f"bd2p{p}_{i}")
                       for i in range(3)] for p in range(P)]
            for p in range(P):
                nc.sync.dma_start(out=wn1t_t[p][:], in_=wn1t_d[p].bitcast(F32R))
                for c in range(C):
                    nc.sync.dma_start(out=we1m_t[p][c][:], in_=we1m_d[p, c])
                nc.sync.dma_start(out=wd1t_t[p][:], in_=wd1t_d[p])
                for c in range(C):
                    nc.sync.dma_start(out=wd2b_t[p][c][:], in_=wd2b_d[p, c])
                for i in range(3):
                    nc.sync.dma_start(out=bd1p_t[p][i][:], in_=bd1p_d[p, i])
                    nc.sync.dma_start(out=bd2p_t[p][i][:], in_=bd2p_d[p, i])
            for c in range(C):
                nc.sync.dma_start(out=bn1c_t[c][:], in_=bn1c_d[c])
                nc.sync.dma_start(out=bn2c_t[c][:], in_=bn2c_d[c])
            upsrc_t = [cp.tile([128, NBK], I32, name=f"upsrc{p}")
                       for p in range(P)]
            updr_t = [cp.tile([128, NBK], BF16, name=f"updr{p}")
                      for p in range(P)]
            for p in range(P):
                nc.scalar.dma_start(out=upsrc_t[p][:], in_=up_src_d[p])
                nc.scalar.dma_start(out=updr_t[p][:], in_=up_dr_d[p])
            bidx_t, bsrel_t, bstart_t, bend_t, degt_t = [], [], [], [], []
            for ph in range(2 * P):
                bi = cp.tile([128, G2 * 8], I16, name=f"bidx{ph}")
                nc.scalar.dma_start(out=bi[:], in_=bidx_d[ph])
                bidx_t.append(bi)
                s = cp.tile([128, G2], BF16, name=f"bsrel{ph}")
                nc.scalar.dma_start(out=s[:], in_=bsrel_d[ph])
                bsrel_t.append(s)
                s0 = cp.tile([128, G2], BF16, name=f"bstart{ph}")
                nc.scalar.dma_start(out=s0[:], in_=bstart_d[ph])
                bstart_t.append(s0)
                s1 = cp.tile([128, G2], BF16, name=f"bend{ph}")
                nc.scalar.dma_start(out=s1[:], in_=bend_d[ph])
                bend_t.append(s1)
                dg = cp.tile([128, NMT], F32, name=f"degt{ph}")
                nc.scalar.dma_start(out=dg[:], in_=degt_d[ph])
                degt_t.append(dg)
            ones_f = cp.tile([1, GRP * 128], F32)
            nc.vector.memset(ones_f[:], 1.0)
            ones_r = cp.tile([1, GRP * 128], F32R)
            nc.vector.tensor_copy(out=ones_r[:], in_=ones_f[:])

            # ======================= UP PHASE =======================
            n_loc_ap = n_loc.ap()
            with tc.tile_pool(name="up_sb", bufs=3) as up, \
                 tc.tile_pool(name="up_g", bufs=32) as upg, \
                 tc.tile_pool(name="up_sb1", bufs=2) as up1, \
                 tc.tile_pool(name="up_ps", bufs=2, space="PSUM") as upp, \
                 tc.tile_pool(name="up_ps1", bufs=1, space="PSUM") as upp1, \
                 tc.tile_pool(name="mlp_ps", bufs=1, space="PSUM") as mpp:
                uqr = [0]
                for g0 in (range(0, NB, GRP) if "up" in stages else []):
                    gb = list(range(g0, min(g0 + GRP, NB)))
                    GW = len(gb) * 128
                    upX = [[up1.tile([FP, GRP * 128], F32R,
                                     name=f"upX{p}_{c}", tag=f"upX{p}_{c}")
                            for c in range(C)] for p in range(P)]
                    for p in range(P):
                        for bi, b in enumerate(gb):
                            kbb = KB[p][b]
                            kb0 = KBOFF[p][b]
                            O8 = up.tile([128, K_UP, 128], BF16, tag="O8")
                            csl0 = slice(kb0, kb0 + kbb)
                            nc.vector.tensor_tensor(
                                out=O8[:, 0:kbb, :],
                                in0=updr_t[p][:, csl0]
                                    .rearrange("a (b c) -> a b c", c=1)
                                    .to_broadcast([128, kbb, 128]),
                                in1=iotab_t[:].rearrange("a (b c) -> a b c",
                                                         b=1)
                                    .to_broadcast([128, kbb, 128]),
                                op=ALU.is_equal)
                            pu = upp.tile([128, CF], F32, tag="pu",
                                          space="PSUM")
                            for kk in range(kbb):
                                col = kb0 + kk
                                G = upg.tile([128, CF], BF16, tag="G")
                                gi = nc.gpsimd.indirect_dma_start(
                                    out=G[:], out_offset=None,
                                    in_=xbf_d[:],
                                    in_offset=bass.IndirectOffsetOnAxis(
                                        ap=upsrc_t[p][:, col:col + 1], axis=0))
                                qi = uqr[0] % 4
                                uqr[0] += 1
                                if qi:
                                    gi.ins.queue = f"qPoolDynamic{qi}"
                                nc.tensor.matmul(out=pu[:],
                                                 lhsT=O8[:, kk, :],
                                                 rhs=G[:], start=(kk == 0),
                                                 stop=(kk == kbb - 1))
                            stg = up.tile([128, CF], F32, tag="stg")
                            nc.scalar.copy(out=stg[:], in_=pu[:])
                            csl = slice(bi * 128, (bi + 1) * 128)
                            for ti in range(3):
                                w = min(128, CF - ti * 128)
                                pt = upp1.tile([128, 128], F32, tag="ptr",
                                               space="PSUM")
                                nc.tensor.transpose(
                                    out=pt[:w, :],
                                    in_=stg[:, ti * 128:ti * 128 + w],
                                    identity=ident_t[:])
                                nc.vector.tensor_copy(
                                    out=upX[p][2 * ti][:, csl],
                                    in_=pt[0:FP, :])
                                if 2 * ti + 1 < C:
                                    nc.vector.tensor_copy(
                                        out=upX[p][2 * ti + 1][:, csl],
                                        in_=pt[FP:2 * FP, :])
                    # ---- nexus MLP over this group ----
                    n1c = [up.tile([FN, GRP * 128], F32R, name=f"n1c{c}",
                                   tag=f"n1c{c}") for c in range(C)]
                    for c in range(C):
                        pn1 = mpp.tile([FN, GRP * 128], F32, tag="pn1",
                                       space="PSUM", bufs=2)
                        for p in range(P):
                            nc.tensor.matmul(
                                out=pn1[:, :GW],
                                lhsT=wn1t_t[p][:, c * FN:(c + 1) * FN],
                                rhs=upX[p][c][:, :GW],
                                start=(p == 0), stop=(p == P - 1))
                        nc.scalar.activation(n1c[c][:, :GW], pn1[:, :GW],
                                             TANH, bias=bn1c_t[c][:])
                    n2s = up.tile([4 * FN, GRP * 128], F32R, tag="n2s")
                    nbt = up.tile([FN + 1, GRP * 128], F32R, tag="nbt")
                    nc.vector.tensor_copy(out=nbt[FN:FN + 1, :],
                                          in_=ones_r[:])
                    for c in range(C):
                        pn2 = mpp.tile([FN, GRP * 128], F32, tag="pn2",
                                       space="PSUM", bufs=2)
                        nc.tensor.matmul(
                            out=pn2[:, :GW],
                            lhsT=wn2t_t[:, c * FN:(c + 1) * FN],
                            rhs=n1c[c][:, :GW], start=True, stop=True)
                        dst = (n2s[c * FN:(c + 1) * FN, :GW] if c < 4
                               else nbt[0:FN, :GW])
                        nc.scalar.activation(dst, pn2[:, :GW],
                                             TANH, bias=bn2c_t[c][:])
                    pbv = mpp.tile([C * P, GRP * 128], F32, tag="misc",
                                   space="PSUM", bufs=1)
                    nc.tensor.matmul(out=pbv[:, :GW], lhsT=wentA_t[:],
                                     rhs=n2s[:, :GW], start=True, stop=False)
                    nc.tensor.matmul(out=pbv[:, :GW], lhsT=wentB_t[:],
                                     rhs=nbt[:, :GW], start=False, stop=True)
                    bt = up.tile([C * P, GRP * 128], F32, tag="bt")
                    nc.scalar.copy(out=bt[:, :GW], in_=pbv[:, :GW])
                    # assemble + store n rows per block
                    for bi, b in enumerate(gb):
                        rows = min(128, M_LOC - b * 128)
                        sl = slice(bi * 128, bi * 128 + 128)
                        tp = mpp.tile([128, CN + C * P], F32,
                                      tag="misc", space="PSUM", bufs=1)
                        nc.tensor.transpose(
                            out=tp[:, 0:4 * FN],
                            in_=n2s[:, sl].bitcast(F32),
                            identity=ident_t[:])
                        nc.tensor.transpose(
                            out=tp[:, 4 * FN:CN],
                            in_=nbt[0:FN, sl].bitcast(F32),
                            identity=ident_t[:FN, :FN])
                        nc.tensor.transpose(
                            out=tp[:, CN:CN + C * P],
                            in_=bt[:, sl],
                            identity=ident_t[:C * P, :C * P])
                        nrow = up.tile([128, NR], F32, tag="nrow")
                        nc.scalar.copy(out=nrow[:, 0:CN + C * P], in_=tp[:])
                        nc.vector.memset(nrow[:, CN + C * P:], 0.0)
                        nc.sync.dma_start(
                            out=n_loc_ap[b * 128:b * 128 + rows, :],
                            in_=nrow[:rows, :])
                    if "ag" in stages:
                        CH = GRP * 128
                        jch = g0 // GRP
                        lo = jch * CH
                        hi = min(lo + CH, M_LOC)
                        if hi > lo:
                            base = NC * CH * jch
                            nrows = NC * (hi - lo)
                            nfv = n_full.ap()[base:base + nrows, :]
                            nc.gpsimd.collective_compute(
                                "AllGather", ALU.bypass,
                                replica_groups=[list(range(NC))],
                                ins=[n_loc_ap[lo:hi, :].opt()],
                                outs=[nfv.opt()])
                            nc.sync.dma_start(
                                out=n_flat.ap()[base:base + nrows, :],
                                in_=n_full.ap()[base:base + nrows, :]
                                .bitcast(BF16))

            # ================= AllGather n (ablation fallback) ==========
            if "ag" in stages and "up" not in stages:
                CH = GRP * 128
                J_AG = _ceil(M_LOC, CH)
                for jch in range(J_AG):
                    lo = jch * CH
                    hi = min(lo + CH, M_LOC)
                    base = NC * CH * jch
                    nrows = NC * (hi - lo)
                    nc.gpsimd.collective_compute(
                        "AllGather", ALU.bypass,
                        replica_groups=[list(range(NC))],
                        ins=[n_loc.ap()[lo:hi, :].opt()],
                        outs=[n_full.ap()[base:base + nrows, :].opt()])
                    nc.sync.dma_start(
                        out=n_flat.ap()[base:base + nrows, :],
                        in_=n_full.ap()[base:base + nrows, :].bitcast(BF16))

            # ============ FUSED A+B+C per (plane, half) ============
            NG = max(10, K_MAX)
            TBMAX = 4
            qrr = [0]
            packs = {}                    # ph -> list of (t0, tbw)
            for ph0 in range(2 * P):
                lst = []
                t = 0
                while t < NMT:
                    tw, gsum = 0, 0
                    while (t + tw < NMT and tw < TBMAX
                           and (tw == 0
                                or gsum + Kt[ph0][t + tw] <= NG)):
                        gsum += Kt[ph0][t + tw]
                        tw += 1
                    lst.append((t, tw))
                    t += tw
                packs[ph0] = lst
            with tc.tile_pool(name="ft_sb", bufs=1) as ftp, \
                 tc.tile_pool(name="ab_sb", bufs=2) as ab, \
                 tc.tile_pool(name="gn_sb", bufs=8) as gnp, \
                 tc.tile_pool(name="b_sb", bufs=2) as sbp, \
                 tc.tile_pool(name="b_ps", bufs=1, space="PSUM") as bps, \
                 tc.tile_pool(name="agg_ps", bufs=1, space="PSUM") as agp, \
                 tc.tile_pool(name="c_sb", bufs=2) as scb, \
                 tc.tile_pool(name="c_ps", bufs=2, space="PSUM") as scp:
                for ph in range(2 * P):
                    p, h = ph // 2, ph % 2
                    # ---- stage A: load x into ft (bf16), aT, a_sb ----
                    ft = [ftp.tile([FP + FN, NHP], BF16, name=f"ft{c}",
                                   tag=f"ft{c}") for c in range(C)]
                    a_sb = ab.tile([128, NMT, 8], BF16, tag="a_sb")
                    if "a" in stages:
                        for c in range(C):
                            if NHP > NH:
                                nc.vector.memset(ft[c][0:FP, NH:], 0.0)
                            nc.sync.dma_start(
                                out=ft[c][0:FP, :NH],
                                in_=xloc_d[p, h, c * FP:(c + 1) * FP, :])
                        with nc.allow_low_precision(reason="invdeg bf16"):
                            nc.vector.reciprocal(
                                out=a_sb[:, :, 5:6],
                                in_=degt_t[ph][:]
                                .rearrange("a (b c) -> a b c", c=1))
                        for t0 in range(0, NMT, 8):
                            tw = min(8, NMT - t0)
                            pa = bps.tile([128, NG, 6], F32, tag="pa8",
                                          space="PSUM")
                            for ti in range(tw):
                                t = t0 + ti
                                tsl = slice(t * 128, (t + 1) * 128)
                                for c in range(C):
                                    nc.tensor.matmul(
                                        out=pa[:, ti, 0:C],
                                        lhsT=ft[c][0:FP, tsl],
                                        rhs=we1m_t[p][c][:],
                                        start=(c == 0), stop=(c == C - 1))
                            nc.vector.tensor_copy(
                                out=a_sb[:, t0:t0 + tw, 0:5],
                                in_=pa[:, 0:tw, 0:C])
                    # ---- stage B ----
                    if "b" in stages:
                        for (t0, tbw) in packs[ph]:
                            g0 = goff[ph][t0]
                            gw = goff[ph][t0 + tbw] - g0
                            gsl = slice(g0, g0 + gw)
                            if gw == 0:
                                tsl0 = slice(t0 * 128, (t0 + tbw) * 128)
                                for c in range(C):
                                    nc.vector.memset(
                                        ft[c][FP:FP + FN, tsl0], 0.0)
                                continue
                            gn = gnp.tile([128, NG, NR], F32, tag="gn")
                            gnb = gn[:].bitcast(BF16)
                            for go in range(0, gw, GMAX):
                                gww = min(GMAX, gw - go)
                                nc.gpsimd.dma_gather(
                                    out_ap=gnb[:, go:go + gww, :],
                                    in_ap=n_flat.ap()[:],
                                    idxs_ap=bidx_t[ph][
                                        :, (g0 + go) * 8:(g0 + go + gww) * 8],
                                    num_idxs=gww * 128,
                                    num_idxs_reg=gww * 128,
                                    elem_size=2 * NR,
                                    single_packet=True,
                                    queue_num=qrr[0] % 4)
                                qrr[0] += 1
                            Oag = sbp.tile([128, NG, 128], BF16, tag="Oag")
                            nc.vector.tensor_tensor(
                                out=Oag[:, :gw, :],
                                in0=bsrel_t[ph][:, gsl]
                                    .rearrange("a (b c) -> a b c", c=1)
                                    .to_broadcast([128, gw, 128]),
                                in1=iotab_t[:].rearrange("a (b c) -> a b c",
                                                         b=1)
                                    .to_broadcast([128, gw, 128]),
                                op=ALU.is_equal)
                            Oge = sbp.tile([128, NG, 128], BF16, tag="Oge")
                            nc.vector.tensor_tensor(
                                out=Oge[:, :gw, :],
                                in0=iotab_t[:].rearrange("a (b c) -> a b c",
                                                         b=1)
                                    .to_broadcast([128, gw, 128]),
                                in1=bstart_t[ph][:, gsl]
                                    .rearrange("a (b c) -> a b c", c=1)
                                    .to_broadcast([128, gw, 128]),
                                op=ALU.is_ge)
                            Obc = sbp.tile([128, NG, 128], BF16, tag="Obc")
                            nc.vector.tensor_tensor(
                                out=Obc[:, :gw, :],
                                in0=iotab_t[:].rearrange("a (b c) -> a b c",
                                                         b=1)
                                    .to_broadcast([128, gw, 128]),
                                in1=bend_t[ph][:, gsl]
                                    .rearrange("a (b c) -> a b c", c=1)
                                    .to_broadcast([128, gw, 128]),
                                op=ALU.is_lt)
                            nc.vector.tensor_tensor(
                                out=Obc[:, :gw, :], in0=Obc[:, :gw, :],
                                in1=Oge[:, :gw, :], op=ALU.mult)
                            pa8 = bps.tile([128, NG, 6], F32, tag="pa8",
                                           space="PSUM")
                            for ti in range(tbw):
                                t = t0 + ti
                                for jj2 in range(Kt[ph][t]):
                                    jj = goff[ph][t] - g0 + jj2
                                    nc.tensor.matmul(
                                        out=pa8[:, jj, :],
                                        lhsT=Obc[:, jj, :],
                                        rhs=a_sb[:, t, 0:6],
                                        start=True, stop=True)
                            lg = sbp.tile([128, NG, C], F32, tag="lg")
                            nc.vector.tensor_tensor(
                                out=lg[:, :gw, :], in0=pa8[:, :gw, 0:5],
                                in1=gn[:, :gw, CN + p:CN + p
                                       + (C - 1) * P + 1:P],
                                op=ALU.add)
                            ex = sbp.tile([128, NG, C], F32, tag="ex")
                            nc.scalar.activation(ex[:, :gw, :], lg[:, :gw, :],
                                                 EXP)
                            sm = sbp.tile([128, NG], F32, tag="sm")
                            nc.vector.tensor_reduce(
                                out=sm[:, :gw], in_=ex[:, :gw, :],
                                axis=mybir.AxisListType.X, op=ALU.add)
                            nc.vector.reciprocal(out=sm[:, :gw],
                                                 in_=sm[:, :gw])
                            nc.vector.tensor_tensor(
                                out=sm[:, :gw], in0=sm[:, :gw],
                                in1=pa8[:, :gw, 5], op=ALU.mult)
                            nc.vector.tensor_tensor(
                                out=ex[:, :gw, :], in0=ex[:, :gw, :],
                                in1=sm[:, :gw].rearrange("a (b c) -> a b c",
                                                         c=1)
                                    .to_broadcast([128, gw, C]),
                                op=ALU.mult)
                            msg = sbp.tile([128, NG, CN], BF16, tag="msg")
                            nc.vector.tensor_tensor(
                                out=msg[:, :gw, :].rearrange(
                                    "a b (c f) -> a b c f", f=FN),
                                in0=gn[:, :gw, 0:CN].rearrange(
                                    "a b (c f) -> a b c f", f=FN),
                                in1=ex[:, :gw, :].rearrange(
                                    "a b (c d) -> a b c d", d=1)
                                    .to_broadcast([128, gw, C, FN]),
                                op=ALU.mult)
                            psA = agp.tile([128, TBMAX * 128], F32,
                                           tag="psA", space="PSUM")
                            psB = agp.tile([FN, TBMAX * 128], F32,
                                           tag="psB", space="PSUM")
                            nzw = 0
                            for ti in range(tbw):
                                t = t0 + ti
                                kt = Kt[ph][t]
                                if kt == 0:
                                    break
                                nzw += 1
                                bsl = slice(ti * 128, (ti + 1) * 128)
                                for jj2 in range(kt):
                                    jj = goff[ph][t] - g0 + jj2
                                    nc.tensor.matmul(
                                        out=psA[:, bsl],
                                        lhsT=msg[:, jj, 0:128],
                                        rhs=Oag[:, jj, :],
                                        start=(jj2 == 0),
                                        stop=(jj2 == kt - 1))
                                for jj2 in range(kt):
                                    jj = goff[ph][t] - g0 + jj2
                                    nc.tensor.matmul(
                                        out=psB[:, bsl],
                                        lhsT=msg[:, jj, 128:CN],
                                        rhs=Oag[:, jj, :],
                                        start=(jj2 == 0),
                                        stop=(jj2 == kt - 1))
                            csl2 = slice(t0 * 128, (t0 + nzw) * 128)
                            if nzw > 0:
                                for c in range(4):
                                    nc.scalar.copy(
                                        out=ft[c][FP:FP + FN, csl2],
                                        in_=psA[c * FN:(c + 1) * FN,
                                                0:nzw * 128])
                                nc.scalar.copy(
                                    out=ft[4][FP:FP + FN, csl2],
                                    in_=psB[:, 0:nzw * 128])
                            if nzw < tbw:
                                zsl = slice((t0 + nzw) * 128,
                                            (t0 + tbw) * 128)
                                for c in range(C):
                                    nc.vector.memset(
                                        ft[c][FP:FP + FN, zsl], 0.0)
                    # ---- stage C: down MLP (bf16, class pairs) ----
                    if "c" in stages:
                        for ch0 in range(0, NHP, CHW):
                            cw = min(CHW, NHP - ch0)
                            csl = slice(ch0, ch0 + cw)
                            for pi, pr in enumerate(PAIRS):
                                rows = len(pr) * FP
                                hps = scp.tile([2 * FP, CHW], F32, tag="hps",
                                               space="PSUM")
                                for i, c in enumerate(pr):
                                    nc.tensor.matmul(
                                        out=hps[i * FP:(i + 1) * FP, :cw],
                                        lhsT=wd1t_t[p][:, c * FP:(c + 1) * FP],
                                        rhs=ft[c][:, csl],
                                        start=True, stop=True)
                                ht = scb.tile([2 * FP, CHW], BF16, tag="ht")
                                nc.scalar.activation(
                                    ht[0:rows, :cw], hps[0:rows, :cw],
                                    TANH, bias=bd1p_t[p][pi][0:rows])
                                ops_ = scp.tile([2 * FP, CHW], F32, tag="ops",
                                                space="PSUM")
                                for i, c in enumerate(pr):
                                    nc.tensor.matmul(
                                        out=ops_[i * FP:(i + 1) * FP, :cw],
                                        lhsT=wd2b_t[p][c][
                                            i * FP:(i + 1) * FP, :],
                                        rhs=ht[i * FP:(i + 1) * FP, :cw],
                                        start=True, stop=True)
                                ot = scb.tile([2 * FP, CHW], F32, tag="ot")
                                nc.scalar.activation(
                                    ot[0:rows, :cw], ops_[0:rows, :cw],
                                    TANH, bias=bd2p_t[p][pi][0:rows])
                                for i, c in enumerate(pr):
                                    nc.sync.dma_start(
                                        out=out_d[p, h, c, :, csl],
                                        in_=ot[i * FP:(i + 1) * FP, :cw])

    nc.compile()
    return nc


_CACHE = {}


def _get_compiled(inputs, cfg):
    in_maps, meta = host_prep(inputs, cfg)
    key = (meta["K_UP"], meta["Kt"], tuple(sorted(cfg.items())))
    if key not in _CACHE:
        _CACHE[key] = build_kernel(meta)
    return _CACHE[key], in_maps, meta


def assemble_output(results, meta):
    cfg = meta["cfg"]
    P, N, C, FP, NC = (cfg[k] for k in ("P", "N", "C", "FP", "NC"))
    NH = meta["NH"]
    pc, phh, pl = meta["perm_core"], meta["perm_h"], meta["perm_loc"]
    arr = np.stack([np.asarray(results[k]["outT"])[:, :, :, :, :NH]
                    for k in range(NC)])      # [NC, P, 2, C, FP, NH]
    out = np.empty((P, N, C, FP), np.float32)
    for p in range(P):
        out[p] = arr[pc[p], p, phh[p], :, :, pl[p]]
    return np.ascontiguousarray(out)


def kernel(**inputs):
    from concourse.bass_utils import run_bass_kernel_spmd
    cfg = CFG_FULL
    nc, in_maps, meta = _get_compiled(inputs, cfg)
    res = run_bass_kernel_spmd(nc, in_maps, list(range(cfg["NC"])))
    return assemble_output(res.results, meta)
